# revision 17
# baseline (speedup 1.0000x reference)
"""DGCNN semantic-segmentation kernel for 8x Trainium2 NeuronCores.

Strategy: data-parallel over batch. B=4 samples; core c processes sample c%4
end-to-end (cores 4-7 duplicate work so one SPMD program runs everywhere);
host takes outputs from cores 0-3. Everything for one sample stays on one
core: no cross-core communication.

Per-sample pipeline (all on device):
  3x EdgeConv blocks: kNN (fp16 hi/lo split K=13 distance matmul, ~1e-6
  accurate) -> top-20 selection (pack 8-bit chunk-local index into low
  mantissa bits, top-8 per 256-chunk via DVE max8, refine via
  max8/match_replace, recover columns arithmetically) -> streamed gather via
  GPSIMD ap_gather -> folded 1x1 conv + instance-norm + leaky-relu ->
  second conv (fp32 matmul) streamed with running max over k (max commutes
  with the monotone normalize+lrelu since scale > 0). Then the global head
  (w6..w9) with in1d norms, mostly bf16 matmuls.

Inter-block layout: packed [128, 2048] fp32 - partition p<64 holds channel p
for points n<2048, partition 64+p holds channel p for n>=2048.
"""

import sys

if "/opt/trn_rl_repo" not in sys.path:
    sys.path.insert(0, "/opt/trn_rl_repo")

import numpy as np

N = 4096
NT = 32          # row tiles of 128 for the distance/selection loop
HB = 2048
KNN = 20
NK = KNN * HB    # free size of the (virtual) packed h tensor per partition
CHUNK = 256
NEG = -3.0e38

_CACHE = {}


def _build_program():
    import concourse.bacc as bacc
    import concourse.tile as tile
    from concourse import mybir
    from contextlib import ExitStack

    F32 = mybir.dt.float32
    F32R = mybir.dt.float32r
    F16 = mybir.dt.float16
    BF16 = mybir.dt.bfloat16
    U32 = mybir.dt.uint32
    U16 = mybir.dt.uint16
    I16 = mybir.dt.int16
    AF = mybir.ActivationFunctionType
    ALU = mybir.AluOpType
    AX = mybir.AxisListType

    nc = bacc.Bacc("TRN2", target_bir_lowering=False, debug=False, num_devices=8)

    def din(name, shape, dt=F32):
        return nc.dram_tensor(name, shape, dt, kind="ExternalInput").ap()

    xt_d = din("xt", [4, N])
    waT_d = [din("waT1", [4, 64]), din("waT3", [64, 64]), din("waT5", [64, 64])]
    wbT_d = [din("wbT1", [4, 64]), din("wbT3", [64, 64]), din("wbT5", [64, 64])]
    wcT_d = [din("w2T", [64, 64]), din("w4T", [64, 64])]
    w6T_d = [din(f"w6T_{k}", [64, 1024], BF16) for k in range(3)]
    w7gT_d = [din(f"w7gT_{k}", [128, 512], BF16) for k in range(8)]
    w7xT_d = [din(f"w7xT_{k}", [64, 512], BF16) for k in range(3)]
    w8T_d = [din(f"w8T_{k}", [128, 256], BF16) for k in range(4)]
    w9T_d = [din(f"w9T_{k}", [128, 2], BF16) for k in range(2)]
    iota256_d = din("iota256", [128, CHUNK], U32)
    maskc_d = din("maskc", [128, 1], U32)
    magic_d = din("magic", [128, 1], U32)
    id2_d = din("id2", [2, 2])

    out_d = nc.dram_tensor("out", [N, 2], F32, kind="ExternalOutput").ap()

    with tile.TileContext(nc) as tc, ExitStack() as ctx:
        wpool = ctx.enter_context(tc.tile_pool(name="wpool", bufs=1))
        xpool = ctx.enter_context(tc.tile_pool(name="xpool", bufs=1))
        stpool = ctx.enter_context(tc.tile_pool(name="stpool", bufs=1))
        pp = ctx.enter_context(tc.tile_pool(name="pp", bufs=2, space="PSUM"))
        ectx = ExitStack()
        abpool = ectx.enter_context(tc.tile_pool(name="abpool", bufs=1))
        gpool = ectx.enter_context(tc.tile_pool(name="gpool", bufs=1))
        dpool = ectx.enter_context(tc.tile_pool(name="dpool", bufs=2))
        selpool = ectx.enter_context(tc.tile_pool(name="selpool", bufs=4))
        idxpool = ectx.enter_context(tc.tile_pool(name="idxpool", bufs=1))
        chpool = ectx.enter_context(tc.tile_pool(name="chpool", bufs=1))

        def load(pool, ap_d, shape, dt=F32, dup64=False):
            rows = shape[0]
            tshape = [128, shape[1]] if dup64 else shape
            t = pool.tile(tshape, dt, tag=ap_d.tensor.name, name=ap_d.tensor.name + "_sb")
            nc.sync.dma_start(t[0:rows, :], ap_d)
            if dup64:
                nc.sync.dma_start(t[64:64 + rows, :], ap_d)
            return t

        waT = [load(wpool, waT_d[0], [4, 64], dup64=True),
               load(wpool, waT_d[1], [64, 64], dup64=True),
               load(wpool, waT_d[2], [64, 64], dup64=True)]
        wbT = [load(wpool, wbT_d[0], [4, 64], dup64=True),
               load(wpool, wbT_d[1], [64, 64], dup64=True),
               load(wpool, wbT_d[2], [64, 64], dup64=True)]
        wcT = [load(wpool, wcT_d[0], [64, 64], dup64=True),
               load(wpool, wcT_d[1], [64, 64], dup64=True), None]
        w6T = [load(wpool, a, [64, 1024], BF16, dup64=True) for a in w6T_d]
        w7gT = [load(wpool, a, [128, 512], BF16) for a in w7gT_d]
        w7xT = [load(wpool, a, [64, 512], BF16, dup64=True) for a in w7xT_d]
        w8T = [load(wpool, a, [128, 256], BF16) for a in w8T_d]
        w9T = [load(wpool, a, [128, 2], BF16) for a in w9T_d]
        iota256 = load(wpool, iota256_d, [128, CHUNK], U32)
        maskc = load(wpool, maskc_d, [128, 1], U32)
        magic = load(wpool, magic_d, [128, 1], U32)
        id2 = load(wpool, id2_d, [2, 2])

        xt_sb = gpool.tile([4, N], F32, tag="ya_dup", name="xt_sb")
        nc.sync.dma_start(xt_sb[:], xt_d)
        x_p = [xpool.tile([128, HB], F32, tag=f"x{i}_p", name=f"x{i}_p") for i in range(4)]
        nc.sync.dma_start(x_p[0][0:4, :], xt_sb[:, 0:HB])
        nc.sync.dma_start(x_p[0][64:68, :], xt_sb[:, HB:N])

        def small(tag, shape=(128, 1), dt=F32):
            return stpool.tile(list(shape), dt, tag=tag, name=tag)

        def ts(out, in0, s1, op0, s2=None, op1=None):
            if op1 is None:
                nc.vector.tensor_scalar(out, in0, s1, None, op0)
            else:
                nc.vector.tensor_scalar(out, in0, s1, s2, op0, op1)

        def rsqrt_inplace(y, t_in, rows):
            b = small("rs_b", (rows, 1), U32)
            ts(b[:], t_in.bitcast(U32), 1, ALU.logical_shift_right)
            nc.vector.tensor_tensor(y.bitcast(U32), magic[0:rows, :], b[:], ALU.subtract)
            for _ in range(2):
                u = small("rs_u", (rows, 1))
                nc.vector.tensor_tensor(u[:], y, y, ALU.mult)
                nc.vector.tensor_tensor(u[:], u[:], t_in, ALU.mult)
                ts(u[:], u[:], -0.5, ALU.mult, 1.5, ALU.add)
                nc.vector.tensor_tensor(y, y, u[:], ALU.mult)

        def scale_bias_from_mv(mv):
            """mv [128,2] per-partition (mean, var); rows p/p+64 are halves of one
            channel. Returns s128, b128 [128,1] with normalized = s*x + b."""
            mvb = small("st_mvb", (64, 2))
            nc.sync.dma_start(mvb[:], mv[64:128, :])
            m = small("st_m", (64, 1)); v = small("st_v", (64, 1))
            dm = small("st_dm", (64, 1))
            nc.vector.tensor_tensor(m[:], mv[0:64, 0:1], mvb[:, 0:1], ALU.add)
            ts(m[:], m[:], 0.5, ALU.mult)
            nc.vector.tensor_tensor(v[:], mv[0:64, 1:2], mvb[:, 1:2], ALU.add)
            nc.vector.tensor_tensor(dm[:], mv[0:64, 0:1], mvb[:, 0:1], ALU.subtract)
            nc.vector.tensor_tensor(dm[:], dm[:], dm[:], ALU.mult)
            ts(v[:], v[:], 0.5, ALU.mult)
            ts(dm[:], dm[:], 0.25, ALU.mult)
            nc.vector.tensor_tensor(v[:], v[:], dm[:], ALU.add)
            ts(v[:], v[:], 1e-5, ALU.add)
            s = small("st_s", (64, 1))
            rsqrt_inplace(s[:], v[:], 64)
            bb = small("st_bb", (64, 1))
            nc.vector.tensor_tensor(bb[:], m[:], s[:], ALU.mult)
            ts(bb[:], bb[:], -1.0, ALU.mult)
            s128 = small("st_s128"); b128 = small("st_b128")
            nc.vector.tensor_copy(s128[0:64, :], s[:])
            nc.vector.tensor_copy(b128[0:64, :], bb[:])
            nc.sync.dma_start(s128[64:128, :], s[:])
            nc.sync.dma_start(b128[64:128, :], bb[:])
            return s128, b128

        def mv_from_sums(ssum, ssq, count):
            """[128, w] partial sums -> mv [128, 2] (mean, var per partition)."""
            mv = small("sm_mv", (128, 2))
            nc.vector.tensor_reduce(mv[:, 0:1], ssum[:], axis=AX.X, op=ALU.add)
            nc.vector.tensor_reduce(mv[:, 1:2], ssq[:], axis=AX.X, op=ALU.add)
            ts(mv[:, 0:1], mv[:, 0:1], 1.0 / count, ALU.mult)
            ts(mv[:, 1:2], mv[:, 1:2], 1.0 / count, ALU.mult)
            m2 = small("sm_m2")
            nc.vector.tensor_tensor(m2[:], mv[:, 0:1], mv[:, 0:1], ALU.mult)
            nc.vector.tensor_tensor(mv[:, 1:2], mv[:, 1:2], m2[:], ALU.subtract)
            return mv

        # ---------------- EdgeConv block ----------------
        def edge_block(bi, xin_p, C):
            has_conv2 = bi < 2

            # distance operand prep: pieces computed at partition base 0 (ACT
            # alignment rule), assembled into aT/bT via DMAs.
            sq = dpool.tile([128, HB], F32, tag="dpk", name="sq")
            nc.scalar.activation(sq[0:3, :], xin_p[0:3, :], AF.Square)
            nc.scalar.activation(sq[64:67, :], xin_p[64:67, :], AF.Square)
            ones3 = abpool.tile([128, 1], F32, tag="ones3")
            nc.gpsimd.memset(ones3[:], 1.0)

            src4 = gpool.tile([4, N], F32, tag="ya_dup", name="src4")
            for h in range(2):
                psx = pp.tile([128, HB], F32, tag="pp")
                for j in range(4):
                    nc.tensor.matmul(
                        psx[0:1, 512 * j:512 * (j + 1)],
                        ones3[64 * h:64 * h + 3, 0:1],
                        sq[64 * h:64 * h + 3, 512 * j:512 * (j + 1)])
                nc.scalar.copy(src4[0:1, HB * h:HB * (h + 1)], psx[0:1, 0:HB])
                nc.sync.dma_start(src4[1:4, HB * h:HB * (h + 1)], xin_p[64 * h:64 * h + 3, :])

            hi4 = gpool.tile([4, N], F16, tag="M", name="hi4")
            lo4 = gpool.tile([4, N], F16, tag="yb_p", name="lo4")
            nc.scalar.copy(hi4[:], src4[:])
            nc.vector.scalar_tensor_tensor(
                lo4[:], hi4[:], -1.0, src4[:], ALU.mult, ALU.add)
            nh4 = dpool.tile([4, N], F16, tag="dpk", name="nh4")
            nl4 = dpool.tile([4, N], F16, tag="dpk", name="nl4")
            nc.scalar.mul(nh4[:], hi4[:], -1.0)
            nc.scalar.mul(nl4[:], lo4[:], -1.0)

            # aT rows: [1, 1, -xxh, -xxl, 2ph(3), 2ph(3), 2pl(3)]
            # bT rows: [-xxh, -xxl, 1, 1, ph(3), pl(3), ph(3)]
            aT = abpool.tile([16, N], F16, tag="aT")
            bT = abpool.tile([16, N], F16, tag="bT")
            nc.gpsimd.memset(aT[0:2, :], 1.0)
            nc.sync.dma_start(bT[2:4, :], aT[0:2, :])
            nc.sync.dma_start(aT[2:3, :], nh4[0:1, :])
            nc.sync.dma_start(aT[3:4, :], nl4[0:1, :])
            nc.sync.dma_start(bT[0:1, :], nh4[0:1, :])
            nc.sync.dma_start(bT[1:2, :], nl4[0:1, :])
            h2x = dpool.tile([4, N], F16, tag="dpk", name="h2x")
            l2x = dpool.tile([4, N], F16, tag="dpk", name="l2x")
            nc.scalar.mul(h2x[:], hi4[:], 2.0)
            nc.scalar.mul(l2x[:], lo4[:], 2.0)
            nc.sync.dma_start(aT[4:7, :], h2x[1:4, :])
            nc.sync.dma_start(aT[7:10, :], h2x[1:4, :])
            nc.sync.dma_start(aT[10:13, :], l2x[1:4, :])
            nc.sync.dma_start(bT[4:7, :], hi4[1:4, :])
            nc.sync.dma_start(bT[7:10, :], lo4[1:4, :])
            nc.sync.dma_start(bT[10:13, :], hi4[1:4, :])

            # ya (duplicated to both partition halves, full n) and yb (packed)
            ya_dup = gpool.tile([128, N], F32, tag="ya_dup")
            yb_p = gpool.tile([128, HB], F32, tag="yb_p")
            for dsth in range(2):
                po = 64 * dsth
                psy = pp.tile([128, HB], F32, tag="pp")
                for srch in range(2):
                    for j in range(4):
                        sl = slice(512 * j, 512 * (j + 1))
                        nc.tensor.matmul(
                            psy[po:po + 64, sl],
                            waT[bi][64 * srch:64 * srch + C, :],
                            xin_p[64 * srch:64 * srch + C, sl])
                    nc.scalar.copy(
                        ya_dup[po:po + 64, HB * srch:HB * (srch + 1)],
                        psy[po:po + 64, 0:HB])
                psb = pp.tile([128, HB], F32, tag="pp")
                for j in range(4):
                    sl = slice(512 * j, 512 * (j + 1))
                    nc.tensor.matmul(
                        psb[po:po + 64, sl],
                        wbT[bi][64 * dsth:64 * dsth + C, :],
                        xin_p[64 * dsth:64 * dsth + C, sl])
                nc.scalar.copy(
                    yb_p[po:po + 64, :].rearrange(
                        "p (g t q) -> p t g q", g=8, t=16, q=16),
                    psb[po:po + 64, 0:HB].rearrange(
                        "p (t g q) -> p t g q", t=16, g=8, q=16))

            # ---- distance + selection ----
            colbuf = idxpool.tile([128, 768], U16, tag="colbuf")
            posall = idxpool.tile([128, 768], U16, tag="posall")
            v24all = idxpool.tile([128, 768], F32, tag="v24all")
            for t in range(NT):
                lhs = aT[0:13, 128 * t:128 * (t + 1)]
                cand = selpool.tile([128, 128], F32, tag="cand")
                for h in range(2):
                    psd = pp.tile([128, HB], F32, tag="pp")
                    for j in range(4):
                        nc.tensor.matmul(
                            psd[:, 512 * j:512 * (j + 1)], lhs,
                            bT[0:13, HB * h + 512 * j:HB * h + 512 * (j + 1)])
                    dpk = dpool.tile([128, HB], U32, tag="dpk")
                    nc.vector.scalar_tensor_tensor(
                        dpk[:].rearrange("p (a c) -> p a c", c=CHUNK),
                        psd[:].bitcast(U32).rearrange("p (a c) -> p a c", c=CHUNK),
                        maskc[:, :],
                        iota256[:].rearrange("p (a c) -> p a c", a=1).broadcast_to([128, 8, CHUNK]),
                        ALU.bitwise_and, ALU.bitwise_or)
                    for c in range(8):
                        nc.vector.max(
                            cand[:, 64 * h + 8 * c:64 * h + 8 * (c + 1)],
                            dpk[:].bitcast(F32)[:, CHUNK * c:CHUNK * (c + 1)])
                v24 = v24all[:, 24 * t:24 * (t + 1)]
                pos = posall[:, 24 * t:24 * (t + 1)]
                c2 = selpool.tile([128, 128], F32, tag="c2")
                c3 = selpool.tile([128, 128], F32, tag="c3")
                nc.vector.max(v24[:, 0:8], cand[:])
                nc.vector.match_replace(c2[:], v24[:, 0:8], cand[:], NEG)
                nc.vector.max(v24[:, 8:16], c2[:])
                nc.vector.match_replace(c3[:], v24[:, 8:16], c2[:], NEG)
                nc.vector.max(v24[:, 16:24], c3[:])
                nc.vector.max_index(pos[:, 0:8], v24[:, 0:8], cand[:])
                nc.vector.max_index(pos[:, 8:16], v24[:, 8:16], cand[:])
                nc.vector.max_index(pos[:, 16:24], v24[:, 16:24], cand[:])
            # batched column arithmetic: col = (pos>>3)*256 + (v24.bits & 255)
            locb = idxpool.tile([128, 768], U32, tag="locb")
            ts(locb[:], v24all[:].bitcast(U32), 255, ALU.bitwise_and)
            loc16b = idxpool.tile([128, 768], U16, tag="loc16b")
            nc.vector.tensor_copy(loc16b[:], locb[:])
            ts(posall[:], posall[:], 3, ALU.logical_shift_right)
            ts(posall[:], posall[:], 8, ALU.logical_shift_left)
            nc.vector.tensor_tensor(
                colbuf[:].rearrange("p (j t) -> p t j", j=24),
                posall[:].rearrange("p (t j) -> p t j", j=24),
                loc16b[:].rearrange("p (t j) -> p t j", j=24), ALU.add)

            # ---- reformat into per-core wrapped gather index lists ----
            # list (per half): position i = j*2048 + n_loc; stored wrapped-16:
            # partition 16k + (i%16), free i//16 = j*128 + t_loc*8 + g2
            wrapped = idxpool.tile([128, 2560], U16, tag="wrapped")
            for h in range(2):
                for g2 in range(8):
                    src = colbuf[16 * g2:16 * (g2 + 1), :] \
                        .rearrange("p (j t) -> p j t", t=NT)[:, 0:KNN, 16 * h:16 * (h + 1)]
                    dst = wrapped[64 * h:64 * h + 16, :] \
                        .rearrange("p (j g t) -> p j g t", g=8, t=16)[:, :, g2, :]
                    nc.sync.dma_start(dst, src)
                for k in range(1, 4):
                    nc.sync.dma_start(
                        wrapped[64 * h + 16 * k:64 * h + 16 * (k + 1), :],
                        wrapped[64 * h:64 * h + 16, :])

            ya3 = ya_dup[:].rearrange("p (m d) -> p m d", d=1)
            wri = wrapped[:].bitcast(I16)

            # ---- pass 1: streamed gather -> h1 chunks -> bn stats ----
            M = gpool.tile([128, HB], F32, tag="M")
            nc.gpsimd.memset(M[:], NEG)
            h1sum = small("h1sum", (128, KNN))
            h1sq = small("h1sq", (128, KNN))
            sscr1 = chpool.tile([128, HB], F32, tag="sscr", bufs=1, name="sscr1")
            for q in range(KNN):
                gch = chpool.tile([128, HB], F32, tag="gch", bufs=3)
                nc.gpsimd.ap_gather(
                    gch[:], ya3, wri[:, 128 * q:128 * (q + 1)],
                    channels=128, num_elems=N, d=1, num_idxs=HB)
                nc.vector.scalar_tensor_tensor(
                    gch[:], gch[:], 1.0, yb_p[:], ALU.mult, ALU.add,
                    accum_out=h1sum[:, q:q + 1])
                nc.scalar.activation(
                    sscr1[:], gch[:], AF.Square, accum_out=h1sq[:, q:q + 1])
                if not has_conv2:
                    nc.vector.tensor_tensor(M[:], M[:], gch[:], ALU.max)
            mv1 = mv_from_sums(h1sum, h1sq, NK)
            s1, b1 = scale_bias_from_mv(mv1)

            if has_conv2:
                # pass 2: re-gather, normalize+lrelu, conv2, running max + sums
                g1s = small("g1s", (128, KNN))
                ssq = small("h2sq", (128, KNN))
                sscr = chpool.tile([128, HB], F32, tag="sscr", bufs=1)
                for j in range(KNN):
                    gch = chpool.tile([128, HB], F32, tag="gch2", bufs=3)
                    nc.gpsimd.ap_gather(
                        gch[:], ya3, wri[:, 128 * j:128 * (j + 1)],
                        channels=128, num_elems=N, d=1, num_idxs=HB)
                    nc.vector.scalar_tensor_tensor(
                        gch[:], gch[:], 1.0, yb_p[:], ALU.mult, ALU.add)
                    nc.scalar.activation(
                        gch[:], gch[:], AF.Prelu, bias=b1[:, :], scale=s1[:, :],
                        alpha=0.2, accum_out=g1s[:, j:j + 1])
                    psc = pp.tile([128, HB], F32, tag="pp")
                    for h in range(2):
                        for jj in range(4):
                            sl = slice(512 * jj, 512 * (jj + 1))
                            nc.tensor.matmul(
                                psc[64 * h:64 * h + 64, sl],
                                wcT[bi][64 * h:64 * h + 64, :],
                                gch[64 * h:64 * h + 64, sl])
                    nc.vector.tensor_tensor(M[:], M[:], psc[:, 0:HB], ALU.max)
                    nc.scalar.activation(
                        sscr[:], psc[:, 0:HB], AF.Square, accum_out=ssq[:, j:j + 1])
                # sum(h2) per channel-half = W2 @ sum(g) (tiny matmuls)
                gsum = small("gsum", (128, 1))
                nc.vector.tensor_reduce(gsum[:], g1s[:], axis=AX.X, op=ALU.add)
                pss = pp.tile([128, HB], F32, tag="pp")
                nc.tensor.matmul(pss[0:64, 0:1], wcT[bi][0:64, :], gsum[0:64, 0:1])
                nc.tensor.matmul(pss[64:128, 0:1], wcT[bi][64:128, :], gsum[64:128, 0:1])
                ssum = small("h2sum", (128, 1))
                nc.scalar.copy(ssum[:], pss[:, 0:1])
                mv2 = mv_from_sums(ssum, ssq, NK)
                s2, b2 = scale_bias_from_mv(mv2)
                xout_p = x_p[bi + 1]
                nc.scalar.activation(
                    xout_p[:].rearrange("p (t g q) -> p g t q", t=16, g=8, q=16),
                    M[:], AF.Prelu, bias=b2[:, :], scale=s2[:, :], alpha=0.2)
            else:
                xout_p = x_p[bi + 1]
                nc.scalar.activation(
                    xout_p[:].rearrange("p (t g q) -> p g t q", t=16, g=8, q=16),
                    M[:], AF.Prelu, bias=b1[:, :], scale=s1[:, :], alpha=0.2)

            return xout_p

        xp = x_p[0]
        for bi in range(3):
            xp = edge_block(bi, xp, 4 if bi == 0 else 64)
        ectx.close()

        # ---------------- head ----------------
        x1_p, x2_p, x3_p = x_p[1], x_p[2], x_p[3]
        hb_pool = ctx.enter_context(tc.tile_pool(name="hb_pool", bufs=1))
        hu_pool = ctx.enter_context(tc.tile_pool(name="hu_pool", bufs=3))
        xb = []
        for i, xpp in enumerate([x1_p, x2_p, x3_p]):
            t = hb_pool.tile([128, HB], BF16, tag=f"xb{i}")
            nc.scalar.copy(t[:], xpp[:])
            xb.append(t)

        # folded per-ki sums of the concat features (for the matmul sum trick):
        # s64[ki] [64,1] f16, row c = sum_n xcat[ch 64ki + c]
        s64 = []
        for ki in range(3):
            sx = small(f"sx{ki}")
            nc.vector.tensor_reduce(sx[:], xb[ki][:], axis=AX.X, op=ALU.add)
            sxb = small(f"sxb{ki}", (64, 1))
            nc.sync.dma_start(sxb[:], sx[64:128, :])
            sf = small(f"sf{ki}", (64, 1), BF16)
            nc.vector.tensor_tensor(sf[:], sx[0:64, :], sxb[:], ALU.add)
            s64.append(sf)

        def stats_from_sums(ssum, ssq, count):
            """ssum/ssq [128,1] f32 over `count` -> s,b with normalized=s*x+b."""
            m = small("st2_m"); v = small("st2_v")
            ts(m[:], ssum, 1.0 / count, ALU.mult)
            ts(v[:], ssq, 1.0 / count, ALU.mult)
            m2 = small("st2_m2")
            nc.vector.tensor_tensor(m2[:], m[:], m[:], ALU.mult)
            nc.vector.tensor_tensor(v[:], v[:], m2[:], ALU.subtract)
            ts(v[:], v[:], 1e-5, ALU.add)
            s = small("st2_s"); b = small("st2_b")
            rsqrt_inplace(s[:], v[:], 128)
            nc.vector.tensor_tensor(b[:], m[:], s[:], ALU.mult)
            ts(b[:], b[:], -1.0, ALU.mult)
            return s, b

        gvecb = hb_pool.tile([128, 8], BF16, tag="gvecb")
        for g in range(8):
            psS = pp.tile([128, HB], F32, tag="pp")
            for ki in range(3):
                nc.tensor.matmul(
                    psS[:, 0:1], w6T[ki][0:64, 128 * g:128 * (g + 1)],
                    s64[ki][:], start=(ki == 0), stop=(ki == 2))
            sum6 = small("sum6")
            nc.scalar.copy(sum6[:], psS[:, 0:1])
            hq6 = small("hq6", (128, 8))
            M6 = hu_pool.tile([128, 512], F32, tag="m6", bufs=2)
            for h in range(2):
                for ci in range(4):
                    sl = slice(512 * ci, 512 * (ci + 1))
                    ps6 = pp.tile([128, HB], F32, tag="pp")
                    for ki in range(3):
                        nc.tensor.matmul(
                            ps6[:, 0:512],
                            w6T[ki][64 * h:64 * h + 64, 128 * g:128 * (g + 1)],
                            xb[ki][64 * h:64 * h + 64, sl],
                            start=(ki == 0), stop=(ki == 2))
                    cidx = 4 * h + ci
                    sq6scr = hu_pool.tile([128, 512], F32, tag="sqscr", bufs=2)
                    nc.scalar.activation(
                        sq6scr[:], ps6[:, 0:512], AF.Square,
                        accum_out=hq6[:, cidx:cidx + 1])
                    if cidx == 0:
                        nc.vector.tensor_copy(M6[:], ps6[:, 0:512])
                    else:
                        nc.vector.tensor_tensor(M6[:], M6[:], ps6[:, 0:512], ALU.max)
            sq1 = small("sq1")
            nc.vector.tensor_reduce(sq1[:], hq6[:], axis=AX.X, op=ALU.add)
            s, b = stats_from_sums(sum6[:], sq1[:], N)
            m1 = small("m1")
            nc.vector.tensor_reduce(m1[:], M6[:], axis=AX.X, op=ALU.max)
            nc.scalar.activation(
                gvecb[:, g:g + 1], m1[:], AF.Prelu, bias=b[:, :], scale=s[:, :],
                alpha=0.2)

        bias7 = hb_pool.tile([128, 4], F32, tag="bias7")
        ps7b = pp.tile([128, HB], F32, tag="pp")
        for og in range(4):
            for g in range(8):
                nc.tensor.matmul(
                    ps7b[:, og:og + 1],
                    w7gT[g][:, 128 * og:128 * (og + 1)],
                    gvecb[:, g:g + 1],
                    start=(g == 0), stop=(g == 7))
        nc.scalar.copy(bias7[:], ps7b[:, 0:4])

        h7b = []
        h7sum = []
        for og in range(4):
            psS = pp.tile([128, HB], F32, tag="pp")
            for ki in range(3):
                nc.tensor.matmul(
                    psS[:, 0:1], w7xT[ki][0:64, 128 * og:128 * (og + 1)],
                    s64[ki][:], start=(ki == 0), stop=(ki == 2))
            sum7 = small("sum7")
            nc.scalar.copy(sum7[:], psS[:, 0:1])
            # add N * bias7 (the Identity-bias below shifts every element)
            nc.vector.scalar_tensor_tensor(
                sum7[:], bias7[:, og:og + 1], float(N), sum7[:], ALU.mult, ALU.add)
            u7 = hu_pool.tile([128, N], F32, tag="uh", name="u7")
            hq7 = small("hq7", (128, 8))
            for h in range(2):
                for ci in range(4):
                    sl = slice(512 * ci, 512 * (ci + 1))
                    ps7 = pp.tile([128, HB], F32, tag="pp")
                    for ki in range(3):
                        nc.tensor.matmul(
                            ps7[:, 0:512],
                            w7xT[ki][64 * h:64 * h + 64, 128 * og:128 * (og + 1)],
                            xb[ki][64 * h:64 * h + 64, sl],
                            start=(ki == 0), stop=(ki == 2))
                    usl = slice(HB * h + 512 * ci, HB * h + 512 * (ci + 1))
                    nc.scalar.activation(
                        u7[:, usl], ps7[:, 0:512],
                        AF.Identity, bias=bias7[:, og:og + 1])
                    cidx = 4 * h + ci
                    sq7scr = hu_pool.tile([128, 512], F32, tag="sqscr", bufs=2)
                    nc.vector.scalar_tensor_tensor(
                        sq7scr[:], u7[:, usl], 1.0, u7[:, usl], ALU.mult,
                        ALU.mult, accum_out=hq7[:, cidx:cidx + 1])
            sq1 = small("sq1")
            nc.vector.tensor_reduce(sq1[:], hq7[:], axis=AX.X, op=ALU.add)
            s, b = stats_from_sums(sum7[:], sq1[:], N)
            t = hb_pool.tile([128, N], BF16, tag=f"h7b{og}")
            hs = small(f"h7s{og}")
            nc.scalar.activation(
                t[:], u7[:], AF.Prelu, bias=b[:, :], scale=s[:, :], alpha=0.2,
                accum_out=hs[:])
            h7b.append(t)
            h7sum.append(hs)
        h7sum16 = small("h7sum16", (128, 4), BF16)
        for ki in range(4):
            nc.vector.tensor_copy(h7sum16[:, ki:ki + 1], h7sum[ki][:])

        h8b = []
        for og in range(2):
            psS = pp.tile([128, HB], F32, tag="pp")
            for ki in range(4):
                nc.tensor.matmul(
                    psS[:, 0:1], w8T[ki][:, 128 * og:128 * (og + 1)],
                    h7sum16[:, ki:ki + 1], start=(ki == 0), stop=(ki == 3))
            sum8 = small("sum8")
            nc.scalar.copy(sum8[:], psS[:, 0:1])
            u8 = hu_pool.tile([128, N], F32, tag="uh", name="u8")
            hq8 = small("hq8", (128, 8))
            for ci in range(8):
                sl = slice(512 * ci, 512 * (ci + 1))
                ps8 = pp.tile([128, HB], F32, tag="pp")
                for ki in range(4):
                    nc.tensor.matmul(
                        ps8[:, 0:512],
                        w8T[ki][:, 128 * og:128 * (og + 1)],
                        h7b[ki][:, sl],
                        start=(ki == 0), stop=(ki == 3))
                nc.scalar.copy(u8[:, sl], ps8[:, 0:512])
                sq8scr = hu_pool.tile([128, 512], F32, tag="sqscr", bufs=2)
                nc.vector.scalar_tensor_tensor(
                    sq8scr[:], u8[:, sl], 1.0, u8[:, sl], ALU.mult,
                    ALU.mult, accum_out=hq8[:, ci:ci + 1])
            sq1 = small("sq1")
            nc.vector.tensor_reduce(sq1[:], hq8[:], axis=AX.X, op=ALU.add)
            s, b = stats_from_sums(sum8[:], sq1[:], N)
            t = hb_pool.tile([128, N], BF16, tag=f"h8b{og}")
            nc.scalar.activation(t[:], u8[:], AF.Prelu, bias=b[:, :], scale=s[:, :], alpha=0.2)
            h8b.append(t)

        o2 = hu_pool.tile([2, N], F32, tag="uh", name="o2")
        for ci in range(8):
            sl = slice(512 * ci, 512 * (ci + 1))
            ps9 = pp.tile([128, HB], F32, tag="pp")
            for ki in range(2):
                nc.tensor.matmul(
                    ps9[0:2, 0:512],
                    w9T[ki][:], h8b[ki][:, sl],
                    start=(ki == 0), stop=(ki == 1))
            nc.scalar.copy(o2[:, sl], ps9[0:2, 0:512])

        ost = hb_pool.tile([128, 64], F32, tag="ost")
        pst = pp.tile([128, HB], F32, tag="pp")
        for t in range(NT):
            nc.tensor.transpose(
                pst[:, 2 * t:2 * (t + 1)], o2[:, 128 * t:128 * (t + 1)], id2[:])
        nc.scalar.copy(ost[:], pst[:, 0:64])
        nc.sync.dma_start(
            out_d.rearrange("(t p) c -> p t c", p=128),
            ost[:].rearrange("p (t c) -> p t c", c=2))

    nc.finalize()
    return nc


def _shared_inputs(ws):
    import ml_dtypes
    w1, w2, w3, w4, w5, w6, w7, w8, w9 = ws
    f32 = np.float32
    bf16 = ml_dtypes.bfloat16
    d = {}
    for i, w in [(1, w1), (3, w3), (5, w5)]:
        C = w.shape[1] // 2
        d[f"waT{i}"] = np.ascontiguousarray(w[:, :C].T.astype(f32))
        d[f"wbT{i}"] = np.ascontiguousarray((w[:, C:] - w[:, :C]).T.astype(f32))
    d["w2T"] = np.ascontiguousarray(w2.T.astype(f32))
    d["w4T"] = np.ascontiguousarray(w4.T.astype(f32))
    w6t = w6.T.astype(bf16); w7gt = w7[:, :1024].T.astype(bf16)
    w7xt = w7[:, 1024:].T.astype(bf16); w8t = w8.T.astype(bf16)
    w9t = w9.T.astype(bf16)
    for k in range(3):
        d[f"w6T_{k}"] = np.ascontiguousarray(w6t[64 * k:64 * (k + 1)])
        d[f"w7xT_{k}"] = np.ascontiguousarray(w7xt[64 * k:64 * (k + 1)])
    for k in range(8):
        d[f"w7gT_{k}"] = np.ascontiguousarray(w7gt[128 * k:128 * (k + 1)])
    for k in range(4):
        d[f"w8T_{k}"] = np.ascontiguousarray(w8t[128 * k:128 * (k + 1)])
    for k in range(2):
        d[f"w9T_{k}"] = np.ascontiguousarray(w9t[128 * k:128 * (k + 1)])
    d["iota256"] = np.broadcast_to(
        np.arange(CHUNK, dtype=np.uint32)[None, :], (128, CHUNK)).copy()
    d["maskc"] = np.full((128, 1), 0xFFFFFF00, dtype=np.uint32)
    d["magic"] = np.full((128, 1), 0x5F3759DF, dtype=np.uint32)
    d["id2"] = np.eye(2, dtype=f32)
    return d


def _run(inputs, want_debug=False):
    from concourse.bass_utils import run_bass_kernel_spmd

    if "nc" not in _CACHE:
        _CACHE["nc"] = _build_program()
    nc = _CACHE["nc"]

    x = np.asarray(inputs["x"], dtype=np.float32)
    ws = [np.asarray(inputs[f"w{i}"], dtype=np.float32) for i in range(1, 10)]
    shared = _shared_inputs(ws)
    in_maps = []
    for c in range(8):
        m = dict(shared)
        m["xt"] = np.ascontiguousarray(x[c % 4].T.astype(np.float32))
        in_maps.append(m)
    res = run_bass_kernel_spmd(nc, in_maps, list(range(8)))
    out = np.stack([res.results[c]["out"] for c in range(4)])
    if want_debug:
        return out, [res.results[c] for c in range(4)]
    return out


def kernel(**inputs):
    return _run(inputs)



# revision 18
# speedup vs baseline: 1.1738x; 1.1738x over previous
"""DGCNN semantic-segmentation kernel for 8x Trainium2 NeuronCores.

Strategy: 2 cores per sample. Core c handles sample c//2, point-half c%2
(2048 of 4096 points). Per block each core computes kNN rows / gather /
EdgeConv for its own points only; pair collectives supply the global parts:
an AllReduce of instance-norm partial sums and an AllGather of the block
output (the next block needs all candidate points). The head (w6..w9) is
cheap and runs duplicated on both cores over all N points; the host takes
even cores' outputs.

Per-core pipeline per block:
  kNN (fp16 hi/lo split K=13 distance matmul, own 2048 rows x all 4096
  cols) -> top-20 selection (pack 8-bit chunk-local index into low mantissa
  bits, top-8 per 256-chunk via DVE max8, refine, recover columns) ->
  streamed gather via GPSIMD ap_gather -> folded 1x1 conv + instance-norm
  (pair-AllReduced stats) + leaky-relu -> second conv streamed with running
  max over k -> prelu -> AllGather of the new features.

Own-point packed layout: [128, 1024] - partition p<64 holds channel p for
own-local points n<1024, partition 64+p for n>=1024. All-point packed
layout: [128, 2048] likewise split at 2048.
"""

import sys

if "/opt/trn_rl_repo" not in sys.path:
    sys.path.insert(0, "/opt/trn_rl_repo")

import numpy as np

N = 4096
NT = 16          # row tiles of 128 for the distance/selection loop (own pts)
HB = 2048        # free size of all-point packed tiles
PL = 1024        # free size of own-point packed tiles
KNN = 20
NK = KNN * HB    # global per-channel-half element count for in2d stats
CHUNK = 256
NEG = -3.0e38
GRPS = [[0, 1], [2, 3], [4, 5], [6, 7]]

_CACHE = {}


def _build_program():
    import concourse.bacc as bacc
    import concourse.tile as tile
    from concourse import mybir
    from contextlib import ExitStack

    F32 = mybir.dt.float32
    F16 = mybir.dt.float16
    BF16 = mybir.dt.bfloat16
    U32 = mybir.dt.uint32
    U16 = mybir.dt.uint16
    I16 = mybir.dt.int16
    AF = mybir.ActivationFunctionType
    ALU = mybir.AluOpType
    AX = mybir.AxisListType

    nc = bacc.Bacc("TRN2", target_bir_lowering=False, debug=False, num_devices=8)

    def din(name, shape, dt=F32):
        return nc.dram_tensor(name, shape, dt, kind="ExternalInput").ap()

    xt_d = din("xt", [4, N])
    xo_d = din("xo", [4, 2048])
    waT_d = [din("waT1", [4, 64]), din("waT3", [64, 64]), din("waT5", [64, 64])]
    wbT_d = [din("wbT1", [4, 64]), din("wbT3", [64, 64]), din("wbT5", [64, 64])]
    wcT_d = [din("w2T", [64, 64]), din("w4T", [64, 64])]
    w6T_d = [din(f"w6T_{k}", [64, 1024], BF16) for k in range(3)]
    w7gT_d = [din(f"w7gT_{k}", [128, 512], BF16) for k in range(8)]
    w7xT_d = [din(f"w7xT_{k}", [64, 512], BF16) for k in range(3)]
    w8T_d = [din(f"w8T_{k}", [128, 256], BF16) for k in range(4)]
    w9T_d = [din(f"w9T_{k}", [128, 2], BF16) for k in range(2)]
    iota256_d = din("iota256", [128, CHUNK], U32)
    maskc_d = din("maskc", [128, 1], U32)
    magic_d = din("magic", [128, 1], U32)
    id2_d = din("id2", [2, 2])

    out_d = nc.dram_tensor("out", [N, 2], F32, kind="ExternalOutput").ap()

    with tile.TileContext(nc) as tc, ExitStack() as ctx:
        wpool = ctx.enter_context(tc.tile_pool(name="wpool", bufs=1))
        xpool = ctx.enter_context(tc.tile_pool(name="xpool", bufs=1))
        stpool = ctx.enter_context(tc.tile_pool(name="stpool", bufs=1))
        pp = ctx.enter_context(tc.tile_pool(name="pp", bufs=2, space="PSUM"))
        cpool = ctx.enter_context(tc.tile_pool(name="cdram", bufs=2, space="DRAM"))
        ectx = ExitStack()
        abpool = ectx.enter_context(tc.tile_pool(name="abpool", bufs=1))
        gpool = ectx.enter_context(tc.tile_pool(name="gpool", bufs=1))
        dpool = ectx.enter_context(tc.tile_pool(name="dpool", bufs=2))
        selpool = ectx.enter_context(tc.tile_pool(name="selpool", bufs=4))
        idxpool = ectx.enter_context(tc.tile_pool(name="idxpool", bufs=1))
        chpool = ectx.enter_context(tc.tile_pool(name="chpool", bufs=1))

        def load(pool, ap_d, shape, dt=F32, dup64=False):
            rows = shape[0]
            tshape = [128, shape[1]] if dup64 else shape
            t = pool.tile(tshape, dt, tag=ap_d.tensor.name, name=ap_d.tensor.name + "_sb")
            nc.sync.dma_start(t[0:rows, :], ap_d)
            if dup64:
                nc.sync.dma_start(t[64:64 + rows, :], ap_d)
            return t

        waT = [load(wpool, waT_d[0], [4, 64], dup64=True),
               load(wpool, waT_d[1], [64, 64], dup64=True),
               load(wpool, waT_d[2], [64, 64], dup64=True)]
        wbT = [load(wpool, wbT_d[0], [4, 64], dup64=True),
               load(wpool, wbT_d[1], [64, 64], dup64=True),
               load(wpool, wbT_d[2], [64, 64], dup64=True)]
        wcT = [load(wpool, wcT_d[0], [64, 64], dup64=True),
               load(wpool, wcT_d[1], [64, 64], dup64=True), None]
        w6T = [load(wpool, a, [64, 1024], BF16, dup64=True) for a in w6T_d]
        w7gT = [load(wpool, a, [128, 512], BF16) for a in w7gT_d]
        w7xT = [load(wpool, a, [64, 512], BF16, dup64=True) for a in w7xT_d]
        w8T = [load(wpool, a, [128, 256], BF16) for a in w8T_d]
        w9T = [load(wpool, a, [128, 2], BF16) for a in w9T_d]
        iota256 = load(wpool, iota256_d, [128, CHUNK], U32)
        maskc = load(wpool, maskc_d, [128, 1], U32)
        magic = load(wpool, magic_d, [128, 1], U32)
        id2 = load(wpool, id2_d, [2, 2])

        xall = [xpool.tile([128, HB], F32, tag=f"xa{i}", name=f"xa{i}")
                for i in range(4)]
        xown = [xpool.tile([128, PL], F32, tag=f"xw{i}", name=f"xw{i}")
                for i in range(4)]
        nc.sync.dma_start(xall[0][0:4, :], xt_d[:, 0:HB])
        nc.sync.dma_start(xall[0][64:68, :], xt_d[:, HB:N])
        nc.sync.dma_start(xown[0][0:4, :], xo_d[:, 0:PL])
        nc.sync.dma_start(xown[0][64:68, :], xo_d[:, PL:2048])

        def small(tag, shape=(128, 1), dt=F32):
            return stpool.tile(list(shape), dt, tag=tag, name=tag)

        def ts(out, in0, s1, op0, s2=None, op1=None):
            if op1 is None:
                nc.vector.tensor_scalar(out, in0, s1, None, op0)
            else:
                nc.vector.tensor_scalar(out, in0, s1, s2, op0, op1)

        def rsqrt_inplace(y, t_in, rows):
            b = small("rs_b", (rows, 1), U32)
            ts(b[:], t_in.bitcast(U32), 1, ALU.logical_shift_right)
            nc.vector.tensor_tensor(y.bitcast(U32), magic[0:rows, :], b[:], ALU.subtract)
            for _ in range(2):
                u = small("rs_u", (rows, 1))
                nc.vector.tensor_tensor(u[:], y, y, ALU.mult)
                nc.vector.tensor_tensor(u[:], u[:], t_in, ALU.mult)
                ts(u[:], u[:], -0.5, ALU.mult, 1.5, ALU.add)
                nc.vector.tensor_tensor(y, y, u[:], ALU.mult)

        def allreduce2(pair):
            """pair [128,2] f32 local partials -> [128,2] summed over the
            2-core pair (via HBM bounce + AllReduce)."""
            din_t = cpool.tile([128, 2], F32, tag="arin")
            dout_t = cpool.tile([128, 2], F32, tag="arout")
            nc.gpsimd.dma_start(din_t[:], pair[:])
            nc.gpsimd.collective_compute(
                "AllReduce", ALU.add, replica_groups=GRPS,
                ins=[din_t.opt()], outs=[dout_t.opt()])
            res = small("ar_res", (128, 2))
            nc.gpsimd.dma_start(res[:], dout_t[:])
            return res

        def scale_bias_from_mv(mv):
            """mv [128,2] per-partition (mean, var); rows p/p+64 are halves of one
            channel. Returns s128, b128 [128,1] with normalized = s*x + b."""
            mvb = small("st_mvb", (64, 2))
            nc.sync.dma_start(mvb[:], mv[64:128, :])
            m = small("st_m", (64, 1)); v = small("st_v", (64, 1))
            dm = small("st_dm", (64, 1))
            nc.vector.tensor_tensor(m[:], mv[0:64, 0:1], mvb[:, 0:1], ALU.add)
            ts(m[:], m[:], 0.5, ALU.mult)
            nc.vector.tensor_tensor(v[:], mv[0:64, 1:2], mvb[:, 1:2], ALU.add)
            nc.vector.tensor_tensor(dm[:], mv[0:64, 0:1], mvb[:, 0:1], ALU.subtract)
            nc.vector.tensor_tensor(dm[:], dm[:], dm[:], ALU.mult)
            ts(v[:], v[:], 0.5, ALU.mult)
            ts(dm[:], dm[:], 0.25, ALU.mult)
            nc.vector.tensor_tensor(v[:], v[:], dm[:], ALU.add)
            ts(v[:], v[:], 1e-5, ALU.add)
            s = small("st_s", (64, 1))
            rsqrt_inplace(s[:], v[:], 64)
            bb = small("st_bb", (64, 1))
            nc.vector.tensor_tensor(bb[:], m[:], s[:], ALU.mult)
            ts(bb[:], bb[:], -1.0, ALU.mult)
            s128 = small("st_s128"); b128 = small("st_b128")
            nc.vector.tensor_copy(s128[0:64, :], s[:])
            nc.vector.tensor_copy(b128[0:64, :], bb[:])
            nc.sync.dma_start(s128[64:128, :], s[:])
            nc.sync.dma_start(b128[64:128, :], bb[:])
            return s128, b128

        def mv_from_totals(tot):
            """tot [128,2] global (sum, sqsum) -> mv [128,2] (mean, var)."""
            mv = small("sm_mv", (128, 2))
            ts(mv[:, 0:1], tot[:, 0:1], 1.0 / NK, ALU.mult)
            ts(mv[:, 1:2], tot[:, 1:2], 1.0 / NK, ALU.mult)
            m2 = small("sm_m2")
            nc.vector.tensor_tensor(m2[:], mv[:, 0:1], mv[:, 0:1], ALU.mult)
            nc.vector.tensor_tensor(mv[:, 1:2], mv[:, 1:2], m2[:], ALU.subtract)
            return mv

        # ---------------- EdgeConv block ----------------
        def edge_block(bi, xa_p, xo_p, C):
            has_conv2 = bi < 2
            ones3 = abpool.tile([128, 1], F32, tag="ones3")
            nc.gpsimd.memset(ones3[:], 1.0)

            # -- distance operand rows for all candidates (bT) --
            sqa = dpool.tile([128, HB], F32, tag="dpk", name="sqa")
            nc.scalar.activation(sqa[0:3, :], xa_p[0:3, :], AF.Square)
            nc.scalar.activation(sqa[64:67, :], xa_p[64:67, :], AF.Square)
            src4a = gpool.tile([4, N], F32, tag="ya_dup", name="src4a")
            for h in range(2):
                psx = pp.tile([128, HB], F32, tag="pp")
                for j in range(4):
                    nc.tensor.matmul(
                        psx[0:1, 512 * j:512 * (j + 1)],
                        ones3[64 * h:64 * h + 3, 0:1],
                        sqa[64 * h:64 * h + 3, 512 * j:512 * (j + 1)])
                nc.scalar.copy(src4a[0:1, HB * h:HB * (h + 1)], psx[0:1, 0:HB])
                nc.sync.dma_start(src4a[1:4, HB * h:HB * (h + 1)], xa_p[64 * h:64 * h + 3, :])
            hiA = gpool.tile([4, N], F16, tag="hiA", name="hiA")
            loA = gpool.tile([4, N], F16, tag="loA", name="loA")
            nc.scalar.copy(hiA[:], src4a[:])
            nc.vector.scalar_tensor_tensor(
                loA[:], hiA[:], -1.0, src4a[:], ALU.mult, ALU.add)
            nhA = dpool.tile([4, N], F16, tag="dpk", name="nhA")
            nlA = dpool.tile([4, N], F16, tag="dpk", name="nlA")
            nc.scalar.mul(nhA[:], hiA[:], -1.0)
            nc.scalar.mul(nlA[:], loA[:], -1.0)

            # -- distance operand rows for own points (aT) --
            sqo = dpool.tile([128, PL], F32, tag="dpk", name="sqo")
            nc.scalar.activation(sqo[0:3, :], xo_p[0:3, :], AF.Square)
            nc.scalar.activation(sqo[64:67, :], xo_p[64:67, :], AF.Square)
            src4o = abpool.tile([4, 2048], F32, tag="src4o", name="src4o")
            for h in range(2):
                psx = pp.tile([128, HB], F32, tag="pp")
                for j in range(2):
                    nc.tensor.matmul(
                        psx[0:1, 512 * j:512 * (j + 1)],
                        ones3[64 * h:64 * h + 3, 0:1],
                        sqo[64 * h:64 * h + 3, 512 * j:512 * (j + 1)])
                nc.scalar.copy(src4o[0:1, PL * h:PL * (h + 1)], psx[0:1, 0:PL])
                nc.sync.dma_start(src4o[1:4, PL * h:PL * (h + 1)], xo_p[64 * h:64 * h + 3, :])
            hiO = gpool.tile([4, 2048], F16, tag="M", name="hiO")
            loO = gpool.tile([4, 2048], F16, tag="yb_p", name="loO")
            nc.scalar.copy(hiO[:], src4o[:])
            nc.vector.scalar_tensor_tensor(
                loO[:], hiO[:], -1.0, src4o[:], ALU.mult, ALU.add)
            nhO = dpool.tile([4, 2048], F16, tag="dpk", name="nhO")
            nlO = dpool.tile([4, 2048], F16, tag="dpk", name="nlO")
            nc.scalar.mul(nhO[:], hiO[:], -1.0)
            nc.scalar.mul(nlO[:], loO[:], -1.0)

            # aT rows (own): [1, 1, -xxh, -xxl, 2ph(3), 2ph(3), 2pl(3)]
            # bT rows (all): [-xxh, -xxl, 1, 1, ph(3), pl(3), ph(3)]
            aT = abpool.tile([16, 2048], F16, tag="aT")
            bT = abpool.tile([16, N], F16, tag="bT")
            nc.gpsimd.memset(bT[2:4, :], 1.0)
            nc.sync.dma_start(aT[0:2, :], bT[2:4, 0:2048])
            nc.sync.dma_start(aT[2:3, :], nhO[0:1, :])
            nc.sync.dma_start(aT[3:4, :], nlO[0:1, :])
            nc.sync.dma_start(bT[0:1, :], nhA[0:1, :])
            nc.sync.dma_start(bT[1:2, :], nlA[0:1, :])
            h2x = dpool.tile([4, 2048], F16, tag="dpk", name="h2x")
            l2x = dpool.tile([4, 2048], F16, tag="dpk", name="l2x")
            nc.scalar.mul(h2x[:], hiO[:], 2.0)
            nc.scalar.mul(l2x[:], loO[:], 2.0)
            nc.sync.dma_start(aT[4:7, :], h2x[1:4, :])
            nc.sync.dma_start(aT[7:10, :], h2x[1:4, :])
            nc.sync.dma_start(aT[10:13, :], l2x[1:4, :])
            nc.sync.dma_start(bT[4:7, :], hiA[1:4, :])
            nc.sync.dma_start(bT[7:10, :], loA[1:4, :])
            nc.sync.dma_start(bT[10:13, :], hiA[1:4, :])

            # ya (all candidates, duplicated to both partition halves) and
            # yb (own points, gather-ordered packing)
            ya_dup = gpool.tile([128, N], F32, tag="ya_dup")
            yb_p = gpool.tile([128, PL], F32, tag="yb_p")
            for dsth in range(2):
                po = 64 * dsth
                psy = pp.tile([128, HB], F32, tag="pp")
                for srch in range(2):
                    for j in range(4):
                        sl = slice(512 * j, 512 * (j + 1))
                        nc.tensor.matmul(
                            psy[po:po + 64, sl],
                            waT[bi][64 * srch:64 * srch + C, :],
                            xa_p[64 * srch:64 * srch + C, sl])
                    nc.scalar.copy(
                        ya_dup[po:po + 64, HB * srch:HB * (srch + 1)],
                        psy[po:po + 64, 0:HB])
                psb = pp.tile([128, HB], F32, tag="pp")
                for j in range(2):
                    sl = slice(512 * j, 512 * (j + 1))
                    nc.tensor.matmul(
                        psb[po:po + 64, sl],
                        wbT[bi][64 * dsth:64 * dsth + C, :],
                        xo_p[64 * dsth:64 * dsth + C, sl])
                nc.scalar.copy(
                    yb_p[po:po + 64, :].rearrange(
                        "p (g t q) -> p t g q", g=8, t=8, q=16),
                    psb[po:po + 64, 0:PL].rearrange(
                        "p (t g q) -> p t g q", t=8, g=8, q=16))

            # ---- distance + selection over own row tiles ----
            colbuf = idxpool.tile([128, 24 * NT], U16, tag="colbuf")
            posall = idxpool.tile([128, 24 * NT], U16, tag="posall")
            v24all = idxpool.tile([128, 24 * NT], F32, tag="v24all")
            for t in range(NT):
                lhs = aT[0:13, 128 * t:128 * (t + 1)]
                cand = selpool.tile([128, 128], F32, tag="cand")
                for h in range(2):
                    psd = pp.tile([128, HB], F32, tag="pp")
                    for j in range(4):
                        nc.tensor.matmul(
                            psd[:, 512 * j:512 * (j + 1)], lhs,
                            bT[0:13, HB * h + 512 * j:HB * h + 512 * (j + 1)])
                    dpk = dpool.tile([128, HB], U32, tag="dpk")
                    nc.vector.scalar_tensor_tensor(
                        dpk[:].rearrange("p (a c) -> p a c", c=CHUNK),
                        psd[:].bitcast(U32).rearrange("p (a c) -> p a c", c=CHUNK),
                        maskc[:, :],
                        iota256[:].rearrange("p (a c) -> p a c", a=1).broadcast_to([128, 8, CHUNK]),
                        ALU.bitwise_and, ALU.bitwise_or)
                    for c in range(8):
                        nc.vector.max(
                            cand[:, 64 * h + 8 * c:64 * h + 8 * (c + 1)],
                            dpk[:].bitcast(F32)[:, CHUNK * c:CHUNK * (c + 1)])
                v24 = v24all[:, 24 * t:24 * (t + 1)]
                pos = posall[:, 24 * t:24 * (t + 1)]
                c2 = selpool.tile([128, 128], F32, tag="c2")
                c3 = selpool.tile([128, 128], F32, tag="c3")
                nc.vector.max(v24[:, 0:8], cand[:])
                nc.vector.match_replace(c2[:], v24[:, 0:8], cand[:], NEG)
                nc.vector.max(v24[:, 8:16], c2[:])
                nc.vector.match_replace(c3[:], v24[:, 8:16], c2[:], NEG)
                nc.vector.max(v24[:, 16:24], c3[:])
                nc.vector.max_index(pos[:, 0:8], v24[:, 0:8], cand[:])
                nc.vector.max_index(pos[:, 8:16], v24[:, 8:16], cand[:])
                nc.vector.max_index(pos[:, 16:24], v24[:, 16:24], cand[:])
            # batched column arithmetic: col = (pos>>3)*256 + (v24.bits & 255)
            locb = idxpool.tile([128, 24 * NT], U32, tag="locb")
            ts(locb[:], v24all[:].bitcast(U32), 255, ALU.bitwise_and)
            loc16b = idxpool.tile([128, 24 * NT], U16, tag="loc16b")
            nc.vector.tensor_copy(loc16b[:], locb[:])
            ts(posall[:], posall[:], 3, ALU.logical_shift_right)
            ts(posall[:], posall[:], 8, ALU.logical_shift_left)
            nc.vector.tensor_tensor(
                colbuf[:].rearrange("p (j t) -> p t j", j=24),
                posall[:].rearrange("p (t j) -> p t j", j=24),
                loc16b[:].rearrange("p (t j) -> p t j", j=24), ALU.add)

            # ---- reformat into wrapped gather index lists ----
            # list (per own sub-half h2 of 1024 pts): position i = j*1024 + c
            # (c = gather-order column); wrapped-16: partition 64*h2+16k+(i%16),
            # free i//16 = j*64 + g2*8 + t
            wrapped = idxpool.tile([128, KNN * 64], U16, tag="wrapped")
            for h2 in range(2):
                for g2 in range(8):
                    src = colbuf[16 * g2:16 * (g2 + 1), :] \
                        .rearrange("p (j t) -> p j t", t=NT)[:, 0:KNN, 8 * h2:8 * (h2 + 1)]
                    dst = wrapped[64 * h2:64 * h2 + 16, :] \
                        .rearrange("p (j g t) -> p j g t", g=8, t=8)[:, :, g2, :]
                    nc.sync.dma_start(dst, src)
                for k in range(1, 4):
                    nc.sync.dma_start(
                        wrapped[64 * h2 + 16 * k:64 * h2 + 16 * (k + 1), :],
                        wrapped[64 * h2:64 * h2 + 16, :])

            ya3 = ya_dup[:].rearrange("p (m d) -> p m d", d=1)
            wri = wrapped[:].bitcast(I16)

            # ---- pass 1: streamed gather -> h1 chunks -> bn stats ----
            M = gpool.tile([128, PL], F32, tag="Mx")
            nc.gpsimd.memset(M[:], NEG)
            h1sum = small("h1sum", (128, KNN))
            h1sq = small("h1sq", (128, KNN))
            sscr1 = chpool.tile([128, PL], F32, tag="sscr", bufs=1, name="sscr1")
            for q in range(KNN):
                gch = chpool.tile([128, PL], F32, tag="gch", bufs=3)
                nc.gpsimd.ap_gather(
                    gch[:], ya3, wri[:, 64 * q:64 * (q + 1)],
                    channels=128, num_elems=N, d=1, num_idxs=PL)
                nc.vector.scalar_tensor_tensor(
                    gch[:], gch[:], 1.0, yb_p[:], ALU.mult, ALU.add,
                    accum_out=h1sum[:, q:q + 1])
                nc.scalar.activation(
                    sscr1[:], gch[:], AF.Square, accum_out=h1sq[:, q:q + 1])
                if not has_conv2:
                    nc.vector.tensor_tensor(M[:], M[:], gch[:], ALU.max)
            pair1 = small("mv_pair", (128, 2))
            nc.vector.tensor_reduce(pair1[:, 0:1], h1sum[:], axis=AX.X, op=ALU.add)
            nc.vector.tensor_reduce(pair1[:, 1:2], h1sq[:], axis=AX.X, op=ALU.add)
            tot1 = allreduce2(pair1)
            mv1 = mv_from_totals(tot1)
            s1, b1 = scale_bias_from_mv(mv1)

            if has_conv2:
                # pass 2: re-gather, normalize+lrelu, conv2, running max + sums
                g1s = small("g1s", (128, KNN))
                ssq = small("h2sq", (128, KNN))
                sscr = chpool.tile([128, PL], F32, tag="sscr", bufs=1)
                for j in range(KNN):
                    gch = chpool.tile([128, PL], F32, tag="gch2", bufs=3)
                    nc.gpsimd.ap_gather(
                        gch[:], ya3, wri[:, 64 * j:64 * (j + 1)],
                        channels=128, num_elems=N, d=1, num_idxs=PL)
                    nc.vector.scalar_tensor_tensor(
                        gch[:], gch[:], 1.0, yb_p[:], ALU.mult, ALU.add)
                    nc.scalar.activation(
                        gch[:], gch[:], AF.Prelu, bias=b1[:, :], scale=s1[:, :],
                        alpha=0.2, accum_out=g1s[:, j:j + 1])
                    psc = pp.tile([128, HB], F32, tag="pp")
                    for h in range(2):
                        for jj in range(2):
                            sl = slice(512 * jj, 512 * (jj + 1))
                            nc.tensor.matmul(
                                psc[64 * h:64 * h + 64, sl],
                                wcT[bi][64 * h:64 * h + 64, :],
                                gch[64 * h:64 * h + 64, sl])
                    nc.vector.tensor_tensor(M[:], M[:], psc[:, 0:PL], ALU.max)
                    nc.scalar.activation(
                        sscr[:], psc[:, 0:PL], AF.Square, accum_out=ssq[:, j:j + 1])
                pair2 = small("mv_pair", (128, 2))
                nc.vector.tensor_reduce(pair2[:, 0:1], g1s[:], axis=AX.X, op=ALU.add)
                nc.vector.tensor_reduce(pair2[:, 1:2], ssq[:], axis=AX.X, op=ALU.add)
                tot2 = allreduce2(pair2)
                # sum(h2) per channel-half = W2 @ (global sum of g)
                pss = pp.tile([128, HB], F32, tag="pp")
                nc.tensor.matmul(pss[0:64, 0:1], wcT[bi][0:64, :], tot2[0:64, 0:1])
                nc.tensor.matmul(pss[64:128, 0:1], wcT[bi][64:128, :], tot2[64:128, 0:1])
                tot2b = small("tot2b", (128, 2))
                nc.scalar.copy(tot2b[:, 0:1], pss[:, 0:1])
                nc.vector.tensor_copy(tot2b[:, 1:2], tot2[:, 1:2])
                mv2 = mv_from_totals(tot2b)
                sx, bx = scale_bias_from_mv(mv2)
            else:
                sx, bx = s1, b1

            xout_o = xown[bi + 1]
            nc.scalar.activation(
                xout_o[:].rearrange("p (t g q) -> p g t q", t=8, g=8, q=16),
                M[:], AF.Prelu, bias=bx[:, :], scale=sx[:, :], alpha=0.2)

            # AllGather own features -> all-point packed layout for next stage
            gin = cpool.tile([128, PL], F32, tag="agin")
            gout = cpool.tile([256, PL], F32, tag="agout")
            nc.gpsimd.dma_start(gin[:], xout_o[:])
            nc.gpsimd.collective_compute(
                "AllGather", ALU.bypass, replica_groups=GRPS,
                ins=[gin.opt()], outs=[gout.opt()])
            xa_next = xall[bi + 1]
            nc.gpsimd.dma_start(xa_next[0:64, 0:PL], gout[0:64, :])
            nc.gpsimd.dma_start(xa_next[0:64, PL:HB], gout[64:128, :])
            nc.gpsimd.dma_start(xa_next[64:128, 0:PL], gout[128:192, :])
            nc.gpsimd.dma_start(xa_next[64:128, PL:HB], gout[192:256, :])
            return xa_next, xout_o

        xa, xo = xall[0], xown[0]
        for bi in range(3):
            xa, xo = edge_block(bi, xa, xo, 4 if bi == 0 else 64)
        ectx.close()

        # ---------------- head (duplicated, all N points) ----------------
        x1_p, x2_p, x3_p = xall[1], xall[2], xall[3]
        hb_pool = ctx.enter_context(tc.tile_pool(name="hb_pool", bufs=1))
        hu_pool = ctx.enter_context(tc.tile_pool(name="hu_pool", bufs=3))
        xb = []
        for i, xpp in enumerate([x1_p, x2_p, x3_p]):
            t = hb_pool.tile([128, HB], BF16, tag=f"xb{i}")
            nc.scalar.copy(t[:], xpp[:])
            xb.append(t)

        # folded per-ki sums of the concat features (for the matmul sum trick)
        s64 = []
        for ki in range(3):
            sx = small(f"sx{ki}")
            nc.vector.tensor_reduce(sx[:], xb[ki][:], axis=AX.X, op=ALU.add)
            sxb = small(f"sxb{ki}", (64, 1))
            nc.sync.dma_start(sxb[:], sx[64:128, :])
            sf = small(f"sf{ki}", (64, 1), BF16)
            nc.vector.tensor_tensor(sf[:], sx[0:64, :], sxb[:], ALU.add)
            s64.append(sf)

        def stats_from_sums(ssum, ssq, count):
            """ssum/ssq [128,1] f32 over `count` -> s,b with normalized=s*x+b."""
            m = small("st2_m"); v = small("st2_v")
            ts(m[:], ssum, 1.0 / count, ALU.mult)
            ts(v[:], ssq, 1.0 / count, ALU.mult)
            m2 = small("st2_m2")
            nc.vector.tensor_tensor(m2[:], m[:], m[:], ALU.mult)
            nc.vector.tensor_tensor(v[:], v[:], m2[:], ALU.subtract)
            ts(v[:], v[:], 1e-5, ALU.add)
            s = small("st2_s"); b = small("st2_b")
            rsqrt_inplace(s[:], v[:], 128)
            nc.vector.tensor_tensor(b[:], m[:], s[:], ALU.mult)
            ts(b[:], b[:], -1.0, ALU.mult)
            return s, b

        gvecb = hb_pool.tile([128, 8], BF16, tag="gvecb")
        for g in range(8):
            psS = pp.tile([128, HB], F32, tag="pp")
            for ki in range(3):
                nc.tensor.matmul(
                    psS[:, 0:1], w6T[ki][0:64, 128 * g:128 * (g + 1)],
                    s64[ki][:], start=(ki == 0), stop=(ki == 2))
            sum6 = small("sum6")
            nc.scalar.copy(sum6[:], psS[:, 0:1])
            hq6 = small("hq6", (128, 8))
            M6 = hu_pool.tile([128, 512], F32, tag="m6", bufs=2)
            for h in range(2):
                for ci in range(4):
                    sl = slice(512 * ci, 512 * (ci + 1))
                    ps6 = pp.tile([128, HB], F32, tag="pp")
                    for ki in range(3):
                        nc.tensor.matmul(
                            ps6[:, 0:512],
                            w6T[ki][64 * h:64 * h + 64, 128 * g:128 * (g + 1)],
                            xb[ki][64 * h:64 * h + 64, sl],
                            start=(ki == 0), stop=(ki == 2))
                    cidx = 4 * h + ci
                    sq6scr = hu_pool.tile([128, 512], F32, tag="sqscr", bufs=2)
                    nc.scalar.activation(
                        sq6scr[:], ps6[:, 0:512], AF.Square,
                        accum_out=hq6[:, cidx:cidx + 1])
                    if cidx == 0:
                        nc.vector.tensor_copy(M6[:], ps6[:, 0:512])
                    else:
                        nc.vector.tensor_tensor(M6[:], M6[:], ps6[:, 0:512], ALU.max)
            sq1 = small("sq1")
            nc.vector.tensor_reduce(sq1[:], hq6[:], axis=AX.X, op=ALU.add)
            s, b = stats_from_sums(sum6[:], sq1[:], N)
            m1 = small("m1")
            nc.vector.tensor_reduce(m1[:], M6[:], axis=AX.X, op=ALU.max)
            nc.scalar.activation(
                gvecb[:, g:g + 1], m1[:], AF.Prelu, bias=b[:, :], scale=s[:, :],
                alpha=0.2)

        bias7 = hb_pool.tile([128, 4], F32, tag="bias7")
        ps7b = pp.tile([128, HB], F32, tag="pp")
        for og in range(4):
            for g in range(8):
                nc.tensor.matmul(
                    ps7b[:, og:og + 1],
                    w7gT[g][:, 128 * og:128 * (og + 1)],
                    gvecb[:, g:g + 1],
                    start=(g == 0), stop=(g == 7))
        nc.scalar.copy(bias7[:], ps7b[:, 0:4])

        h7b = []
        h7sum = []
        for og in range(4):
            psS = pp.tile([128, HB], F32, tag="pp")
            for ki in range(3):
                nc.tensor.matmul(
                    psS[:, 0:1], w7xT[ki][0:64, 128 * og:128 * (og + 1)],
                    s64[ki][:], start=(ki == 0), stop=(ki == 2))
            sum7 = small("sum7")
            nc.scalar.copy(sum7[:], psS[:, 0:1])
            nc.vector.scalar_tensor_tensor(
                sum7[:], bias7[:, og:og + 1], float(N), sum7[:], ALU.mult, ALU.add)
            u7 = hu_pool.tile([128, N], F32, tag="uh", name="u7")
            hq7 = small("hq7", (128, 8))
            for h in range(2):
                for ci in range(4):
                    sl = slice(512 * ci, 512 * (ci + 1))
                    ps7 = pp.tile([128, HB], F32, tag="pp")
                    for ki in range(3):
                        nc.tensor.matmul(
                            ps7[:, 0:512],
                            w7xT[ki][64 * h:64 * h + 64, 128 * og:128 * (og + 1)],
                            xb[ki][64 * h:64 * h + 64, sl],
                            start=(ki == 0), stop=(ki == 2))
                    usl = slice(HB * h + 512 * ci, HB * h + 512 * (ci + 1))
                    nc.scalar.activation(
                        u7[:, usl], ps7[:, 0:512],
                        AF.Identity, bias=bias7[:, og:og + 1])
                    cidx = 4 * h + ci
                    sq7scr = hu_pool.tile([128, 512], F32, tag="sqscr", bufs=2)
                    nc.vector.scalar_tensor_tensor(
                        sq7scr[:], u7[:, usl], 1.0, u7[:, usl], ALU.mult,
                        ALU.mult, accum_out=hq7[:, cidx:cidx + 1])
            sq1 = small("sq1")
            nc.vector.tensor_reduce(sq1[:], hq7[:], axis=AX.X, op=ALU.add)
            s, b = stats_from_sums(sum7[:], sq1[:], N)
            t = hb_pool.tile([128, N], BF16, tag=f"h7b{og}")
            hs = small(f"h7s{og}")
            nc.scalar.activation(
                t[:], u7[:], AF.Prelu, bias=b[:, :], scale=s[:, :], alpha=0.2,
                accum_out=hs[:])
            h7b.append(t)
            h7sum.append(hs)
        h7sum16 = small("h7sum16", (128, 4), BF16)
        for ki in range(4):
            nc.vector.tensor_copy(h7sum16[:, ki:ki + 1], h7sum[ki][:])

        h8b = []
        for og in range(2):
            psS = pp.tile([128, HB], F32, tag="pp")
            for ki in range(4):
                nc.tensor.matmul(
                    psS[:, 0:1], w8T[ki][:, 128 * og:128 * (og + 1)],
                    h7sum16[:, ki:ki + 1], start=(ki == 0), stop=(ki == 3))
            sum8 = small("sum8")
            nc.scalar.copy(sum8[:], psS[:, 0:1])
            u8 = hu_pool.tile([128, N], F32, tag="uh", name="u8")
            hq8 = small("hq8", (128, 8))
            for ci in range(8):
                sl = slice(512 * ci, 512 * (ci + 1))
                ps8 = pp.tile([128, HB], F32, tag="pp")
                for ki in range(4):
                    nc.tensor.matmul(
                        ps8[:, 0:512],
                        w8T[ki][:, 128 * og:128 * (og + 1)],
                        h7b[ki][:, sl],
                        start=(ki == 0), stop=(ki == 3))
                nc.scalar.copy(u8[:, sl], ps8[:, 0:512])
                sq8scr = hu_pool.tile([128, 512], F32, tag="sqscr", bufs=2)
                nc.vector.scalar_tensor_tensor(
                    sq8scr[:], u8[:, sl], 1.0, u8[:, sl], ALU.mult,
                    ALU.mult, accum_out=hq8[:, ci:ci + 1])
            sq1 = small("sq1")
            nc.vector.tensor_reduce(sq1[:], hq8[:], axis=AX.X, op=ALU.add)
            s, b = stats_from_sums(sum8[:], sq1[:], N)
            t = hb_pool.tile([128, N], BF16, tag=f"h8b{og}")
            nc.scalar.activation(t[:], u8[:], AF.Prelu, bias=b[:, :], scale=s[:, :], alpha=0.2)
            h8b.append(t)

        o2 = hu_pool.tile([2, N], F32, tag="uh", name="o2")
        for ci in range(8):
            sl = slice(512 * ci, 512 * (ci + 1))
            ps9 = pp.tile([128, HB], F32, tag="pp")
            for ki in range(2):
                nc.tensor.matmul(
                    ps9[0:2, 0:512],
                    w9T[ki][:], h8b[ki][:, sl],
                    start=(ki == 0), stop=(ki == 1))
            nc.scalar.copy(o2[:, sl], ps9[0:2, 0:512])

        ost = hb_pool.tile([128, 64], F32, tag="ost")
        pst = pp.tile([128, HB], F32, tag="pp")
        for t in range(32):
            nc.tensor.transpose(
                pst[:, 2 * t:2 * (t + 1)], o2[:, 128 * t:128 * (t + 1)], id2[:])
        nc.scalar.copy(ost[:], pst[:, 0:64])
        nc.sync.dma_start(
            out_d.rearrange("(t p) c -> p t c", p=128),
            ost[:].rearrange("p (t c) -> p t c", c=2))

    nc.finalize()
    return nc


def _shared_inputs(ws):
    import ml_dtypes
    w1, w2, w3, w4, w5, w6, w7, w8, w9 = ws
    f32 = np.float32
    bf16 = ml_dtypes.bfloat16
    d = {}
    for i, w in [(1, w1), (3, w3), (5, w5)]:
        C = w.shape[1] // 2
        d[f"waT{i}"] = np.ascontiguousarray(w[:, :C].T.astype(f32))
        d[f"wbT{i}"] = np.ascontiguousarray((w[:, C:] - w[:, :C]).T.astype(f32))
    d["w2T"] = np.ascontiguousarray(w2.T.astype(f32))
    d["w4T"] = np.ascontiguousarray(w4.T.astype(f32))
    w6t = w6.T.astype(bf16); w7gt = w7[:, :1024].T.astype(bf16)
    w7xt = w7[:, 1024:].T.astype(bf16); w8t = w8.T.astype(bf16)
    w9t = w9.T.astype(bf16)
    for k in range(3):
        d[f"w6T_{k}"] = np.ascontiguousarray(w6t[64 * k:64 * (k + 1)])
        d[f"w7xT_{k}"] = np.ascontiguousarray(w7xt[64 * k:64 * (k + 1)])
    for k in range(8):
        d[f"w7gT_{k}"] = np.ascontiguousarray(w7gt[128 * k:128 * (k + 1)])
    for k in range(4):
        d[f"w8T_{k}"] = np.ascontiguousarray(w8t[128 * k:128 * (k + 1)])
    for k in range(2):
        d[f"w9T_{k}"] = np.ascontiguousarray(w9t[128 * k:128 * (k + 1)])
    d["iota256"] = np.broadcast_to(
        np.arange(CHUNK, dtype=np.uint32)[None, :], (128, CHUNK)).copy()
    d["maskc"] = np.full((128, 1), 0xFFFFFF00, dtype=np.uint32)
    d["magic"] = np.full((128, 1), 0x5F3759DF, dtype=np.uint32)
    d["id2"] = np.eye(2, dtype=f32)
    return d


def _run(inputs, want_debug=False):
    from concourse.bass_utils import run_bass_kernel_spmd

    if "nc" not in _CACHE:
        _CACHE["nc"] = _build_program()
    nc = _CACHE["nc"]

    x = np.asarray(inputs["x"], dtype=np.float32)
    ws = [np.asarray(inputs[f"w{i}"], dtype=np.float32) for i in range(1, 10)]
    shared = _shared_inputs(ws)
    in_maps = []
    for c in range(8):
        s, h = c // 2, c % 2
        xt = np.ascontiguousarray(x[s].T.astype(np.float32))
        m = dict(shared)
        m["xt"] = xt
        m["xo"] = np.ascontiguousarray(xt[:, 2048 * h:2048 * (h + 1)])
        in_maps.append(m)
    res = run_bass_kernel_spmd(nc, in_maps, list(range(8)))
    out = np.stack([res.results[2 * s]["out"] for s in range(4)])
    if want_debug:
        return out, [res.results[c] for c in range(8)]
    return out


def kernel(**inputs):
    return _run(inputs)


# revision 19
# speedup vs baseline: 1.1739x; 1.0001x over previous
"""DGCNN semantic-segmentation kernel for 8x Trainium2 NeuronCores.

Strategy: 2 cores per sample. Core c handles sample c//2, point-half c%2
(2048 of 4096 points). Per block each core computes kNN rows / gather /
EdgeConv for its own points only; pair collectives supply the global parts:
an AllReduce of instance-norm partial sums and an AllGather of the block
output (the next block needs all candidate points). The head (w6..w9) is
cheap and runs duplicated on both cores over all N points; the host takes
even cores' outputs.

Per-core pipeline per block:
  kNN (fp16 hi/lo split K=13 distance matmul, own 2048 rows x all 4096
  cols) -> top-20 selection (pack 8-bit chunk-local index into low mantissa
  bits, top-8 per 256-chunk via DVE max8, refine, recover columns) ->
  streamed gather via GPSIMD ap_gather -> folded 1x1 conv + instance-norm
  (pair-AllReduced stats) + leaky-relu -> second conv streamed with running
  max over k -> prelu -> AllGather of the new features.

Own-point packed layout: [128, 1024] - partition p<64 holds channel p for
own-local points n<1024, partition 64+p for n>=1024. All-point packed
layout: [128, 2048] likewise split at 2048.
"""

import sys

if "/opt/trn_rl_repo" not in sys.path:
    sys.path.insert(0, "/opt/trn_rl_repo")

import numpy as np

N = 4096
NT = 16          # row tiles of 128 for the distance/selection loop (own pts)
HB = 2048        # free size of all-point packed tiles
PL = 1024        # free size of own-point packed tiles
KNN = 20
NK = KNN * HB    # global per-channel-half element count for in2d stats
CHUNK = 256
NEG = -3.0e38
GRPS = [[0, 1], [2, 3], [4, 5], [6, 7]]

_CACHE = {}


def _build_program():
    import concourse.bacc as bacc
    import concourse.tile as tile
    from concourse import mybir
    from contextlib import ExitStack

    F32 = mybir.dt.float32
    F16 = mybir.dt.float16
    BF16 = mybir.dt.bfloat16
    U32 = mybir.dt.uint32
    U16 = mybir.dt.uint16
    I16 = mybir.dt.int16
    AF = mybir.ActivationFunctionType
    ALU = mybir.AluOpType
    AX = mybir.AxisListType

    nc = bacc.Bacc("TRN2", target_bir_lowering=False, debug=False, num_devices=8)

    def din(name, shape, dt=F32):
        return nc.dram_tensor(name, shape, dt, kind="ExternalInput").ap()

    xt_d = din("xt", [4, N])
    xo_d = din("xo", [4, 2048])
    waT_d = [din("waT1", [4, 64]), din("waT3", [64, 64]), din("waT5", [64, 64])]
    wbT_d = [din("wbT1", [4, 64]), din("wbT3", [64, 64]), din("wbT5", [64, 64])]
    wcT_d = [din("w2T", [64, 64]), din("w4T", [64, 64])]
    w6T_d = [din(f"w6T_{k}", [64, 1024], BF16) for k in range(3)]
    w7gT_d = [din(f"w7gT_{k}", [128, 512], BF16) for k in range(8)]
    w7xT_d = [din(f"w7xT_{k}", [64, 512], BF16) for k in range(3)]
    w8T_d = [din(f"w8T_{k}", [128, 256], BF16) for k in range(4)]
    w9T_d = [din(f"w9T_{k}", [128, 2], BF16) for k in range(2)]
    iota256_d = din("iota256", [128, CHUNK], U32)
    maskc_d = din("maskc", [128, 1], U32)
    magic_d = din("magic", [128, 1], U32)
    id2_d = din("id2", [2, 2])

    out_d = nc.dram_tensor("out", [N, 2], F32, kind="ExternalOutput").ap()

    with tile.TileContext(nc) as tc, ExitStack() as ctx:
        wpool = ctx.enter_context(tc.tile_pool(name="wpool", bufs=1))
        xpool = ctx.enter_context(tc.tile_pool(name="xpool", bufs=1))
        stpool = ctx.enter_context(tc.tile_pool(name="stpool", bufs=1))
        pp = ctx.enter_context(tc.tile_pool(name="pp", bufs=2, space="PSUM"))
        cpool = ctx.enter_context(tc.tile_pool(name="cdram", bufs=2, space="DRAM"))
        ectx = ExitStack()
        abpool = ectx.enter_context(tc.tile_pool(name="abpool", bufs=1))
        gpool = ectx.enter_context(tc.tile_pool(name="gpool", bufs=1))
        dpool = ectx.enter_context(tc.tile_pool(name="dpool", bufs=2))
        selpool = ectx.enter_context(tc.tile_pool(name="selpool", bufs=4))
        idxpool = ectx.enter_context(tc.tile_pool(name="idxpool", bufs=1))
        chpool = ectx.enter_context(tc.tile_pool(name="chpool", bufs=1))

        def load(pool, ap_d, shape, dt=F32, dup64=False):
            rows = shape[0]
            tshape = [128, shape[1]] if dup64 else shape
            t = pool.tile(tshape, dt, tag=ap_d.tensor.name, name=ap_d.tensor.name + "_sb")
            nc.sync.dma_start(t[0:rows, :], ap_d)
            if dup64:
                nc.sync.dma_start(t[64:64 + rows, :], ap_d)
            return t

        waT = [load(wpool, waT_d[0], [4, 64], dup64=True),
               load(wpool, waT_d[1], [64, 64], dup64=True),
               load(wpool, waT_d[2], [64, 64], dup64=True)]
        wbT = [load(wpool, wbT_d[0], [4, 64], dup64=True),
               load(wpool, wbT_d[1], [64, 64], dup64=True),
               load(wpool, wbT_d[2], [64, 64], dup64=True)]
        wcT = [load(wpool, wcT_d[0], [64, 64], dup64=True),
               load(wpool, wcT_d[1], [64, 64], dup64=True), None]
        w6T = [load(wpool, a, [64, 1024], BF16, dup64=True) for a in w6T_d]
        w7gT = [load(wpool, a, [128, 512], BF16) for a in w7gT_d]
        w7xT = [load(wpool, a, [64, 512], BF16, dup64=True) for a in w7xT_d]
        w8T = [load(wpool, a, [128, 256], BF16) for a in w8T_d]
        w9T = [load(wpool, a, [128, 2], BF16) for a in w9T_d]
        iota256 = load(wpool, iota256_d, [128, CHUNK], U32)
        maskc = load(wpool, maskc_d, [128, 1], U32)
        magic = load(wpool, magic_d, [128, 1], U32)
        id2 = load(wpool, id2_d, [2, 2])

        xall = [xpool.tile([128, HB], F32, tag=f"xa{i}", name=f"xa{i}")
                for i in range(4)]
        xown = [xpool.tile([128, PL], F32, tag=f"xw{i}", name=f"xw{i}")
                for i in range(4)]
        nc.sync.dma_start(xall[0][0:4, :], xt_d[:, 0:HB])
        nc.sync.dma_start(xall[0][64:68, :], xt_d[:, HB:N])
        nc.sync.dma_start(xown[0][0:4, :], xo_d[:, 0:PL])
        nc.sync.dma_start(xown[0][64:68, :], xo_d[:, PL:2048])

        def small(tag, shape=(128, 1), dt=F32):
            return stpool.tile(list(shape), dt, tag=tag, name=tag)

        def ts(out, in0, s1, op0, s2=None, op1=None):
            if op1 is None:
                nc.vector.tensor_scalar(out, in0, s1, None, op0)
            else:
                nc.vector.tensor_scalar(out, in0, s1, s2, op0, op1)

        def rsqrt_inplace(y, t_in, rows):
            b = small("rs_b", (rows, 1), U32)
            ts(b[:], t_in.bitcast(U32), 1, ALU.logical_shift_right)
            nc.vector.tensor_tensor(y.bitcast(U32), magic[0:rows, :], b[:], ALU.subtract)
            for _ in range(2):
                u = small("rs_u", (rows, 1))
                nc.vector.tensor_tensor(u[:], y, y, ALU.mult)
                nc.vector.tensor_tensor(u[:], u[:], t_in, ALU.mult)
                ts(u[:], u[:], -0.5, ALU.mult, 1.5, ALU.add)
                nc.vector.tensor_tensor(y, y, u[:], ALU.mult)

        def allreduce2(pair):
            """pair [128,2] f32 local partials -> [128,2] summed over the
            2-core pair (via HBM bounce + AllReduce)."""
            din_t = cpool.tile([128, 2], F32, tag="arin")
            dout_t = cpool.tile([128, 2], F32, tag="arout")
            nc.gpsimd.dma_start(din_t[:], pair[:])
            nc.gpsimd.collective_compute(
                "AllReduce", ALU.add, replica_groups=GRPS,
                ins=[din_t.opt()], outs=[dout_t.opt()])
            res = small("ar_res", (128, 2))
            nc.gpsimd.dma_start(res[:], dout_t[:])
            return res

        def scale_bias_from_mv(mv):
            """mv [128,2] per-partition (mean, var); rows p/p+64 are halves of one
            channel. Returns s128, b128 [128,1] with normalized = s*x + b."""
            mvb = small("st_mvb", (64, 2))
            nc.sync.dma_start(mvb[:], mv[64:128, :])
            m = small("st_m", (64, 1)); v = small("st_v", (64, 1))
            dm = small("st_dm", (64, 1))
            nc.vector.tensor_tensor(m[:], mv[0:64, 0:1], mvb[:, 0:1], ALU.add)
            ts(m[:], m[:], 0.5, ALU.mult)
            nc.vector.tensor_tensor(v[:], mv[0:64, 1:2], mvb[:, 1:2], ALU.add)
            nc.vector.tensor_tensor(dm[:], mv[0:64, 0:1], mvb[:, 0:1], ALU.subtract)
            nc.vector.tensor_tensor(dm[:], dm[:], dm[:], ALU.mult)
            ts(v[:], v[:], 0.5, ALU.mult)
            ts(dm[:], dm[:], 0.25, ALU.mult)
            nc.vector.tensor_tensor(v[:], v[:], dm[:], ALU.add)
            ts(v[:], v[:], 1e-5, ALU.add)
            s = small("st_s", (64, 1))
            rsqrt_inplace(s[:], v[:], 64)
            bb = small("st_bb", (64, 1))
            nc.vector.tensor_tensor(bb[:], m[:], s[:], ALU.mult)
            ts(bb[:], bb[:], -1.0, ALU.mult)
            s128 = small("st_s128"); b128 = small("st_b128")
            nc.vector.tensor_copy(s128[0:64, :], s[:])
            nc.vector.tensor_copy(b128[0:64, :], bb[:])
            nc.sync.dma_start(s128[64:128, :], s[:])
            nc.sync.dma_start(b128[64:128, :], bb[:])
            return s128, b128

        def mv_from_totals(tot):
            """tot [128,2] global (sum, sqsum) -> mv [128,2] (mean, var)."""
            mv = small("sm_mv", (128, 2))
            ts(mv[:, 0:1], tot[:, 0:1], 1.0 / NK, ALU.mult)
            ts(mv[:, 1:2], tot[:, 1:2], 1.0 / NK, ALU.mult)
            m2 = small("sm_m2")
            nc.vector.tensor_tensor(m2[:], mv[:, 0:1], mv[:, 0:1], ALU.mult)
            nc.vector.tensor_tensor(mv[:, 1:2], mv[:, 1:2], m2[:], ALU.subtract)
            return mv

        # ---------------- EdgeConv block ----------------
        def edge_block(bi, xa_p, xo_p, C):
            has_conv2 = bi < 2
            ones3 = abpool.tile([128, 1], F32, tag="ones3")
            nc.gpsimd.memset(ones3[:], 1.0)

            # -- distance operand rows for all candidates (bT) --
            sqa = dpool.tile([128, HB], F32, tag="dpk", name="sqa")
            nc.scalar.activation(sqa[0:3, :], xa_p[0:3, :], AF.Square)
            nc.scalar.activation(sqa[64:67, :], xa_p[64:67, :], AF.Square)
            src4a = gpool.tile([4, N], F32, tag="ya_dup", name="src4a")
            for h in range(2):
                psx = pp.tile([128, HB], F32, tag="pp")
                for j in range(4):
                    nc.tensor.matmul(
                        psx[0:1, 512 * j:512 * (j + 1)],
                        ones3[64 * h:64 * h + 3, 0:1],
                        sqa[64 * h:64 * h + 3, 512 * j:512 * (j + 1)])
                nc.scalar.copy(src4a[0:1, HB * h:HB * (h + 1)], psx[0:1, 0:HB])
                nc.sync.dma_start(src4a[1:4, HB * h:HB * (h + 1)], xa_p[64 * h:64 * h + 3, :])
            hiA = gpool.tile([4, N], F16, tag="hiA", name="hiA")
            loA = gpool.tile([4, N], F16, tag="loA", name="loA")
            nc.scalar.copy(hiA[:], src4a[:])
            nc.vector.scalar_tensor_tensor(
                loA[:], hiA[:], -1.0, src4a[:], ALU.mult, ALU.add)
            nhA = dpool.tile([4, N], F16, tag="dpk", name="nhA")
            nlA = dpool.tile([4, N], F16, tag="dpk", name="nlA")
            nc.scalar.mul(nhA[:], hiA[:], -1.0)
            nc.scalar.mul(nlA[:], loA[:], -1.0)

            # -- distance operand rows for own points (aT) --
            sqo = dpool.tile([128, PL], F32, tag="dpk", name="sqo")
            nc.scalar.activation(sqo[0:3, :], xo_p[0:3, :], AF.Square)
            nc.scalar.activation(sqo[64:67, :], xo_p[64:67, :], AF.Square)
            src4o = abpool.tile([4, 2048], F32, tag="src4o", name="src4o")
            for h in range(2):
                psx = pp.tile([128, HB], F32, tag="pp")
                for j in range(2):
                    nc.tensor.matmul(
                        psx[0:1, 512 * j:512 * (j + 1)],
                        ones3[64 * h:64 * h + 3, 0:1],
                        sqo[64 * h:64 * h + 3, 512 * j:512 * (j + 1)])
                nc.scalar.copy(src4o[0:1, PL * h:PL * (h + 1)], psx[0:1, 0:PL])
                nc.sync.dma_start(src4o[1:4, PL * h:PL * (h + 1)], xo_p[64 * h:64 * h + 3, :])
            hiO = gpool.tile([4, 2048], F16, tag="M", name="hiO")
            loO = gpool.tile([4, 2048], F16, tag="yb_p", name="loO")
            nc.scalar.copy(hiO[:], src4o[:])
            nc.vector.scalar_tensor_tensor(
                loO[:], hiO[:], -1.0, src4o[:], ALU.mult, ALU.add)
            nhO = dpool.tile([4, 2048], F16, tag="dpk", name="nhO")
            nlO = dpool.tile([4, 2048], F16, tag="dpk", name="nlO")
            nc.scalar.mul(nhO[:], hiO[:], -1.0)
            nc.scalar.mul(nlO[:], loO[:], -1.0)

            # aT rows (own): [1, 1, -xxh, -xxl, 2ph(3), 2ph(3), 2pl(3)]
            # bT rows (all): [-xxh, -xxl, 1, 1, ph(3), pl(3), ph(3)]
            aT = abpool.tile([16, 2048], F16, tag="aT")
            bT = abpool.tile([16, N], F16, tag="bT")
            onesb = abpool.tile([2, N], F16, tag="onesb")
            nc.gpsimd.memset(onesb[:], 1.0)
            nc.sync.dma_start(bT[2:4, :], onesb[:])
            nc.sync.dma_start(aT[0:2, :], onesb[:, 0:2048])
            nc.sync.dma_start(aT[2:3, :], nhO[0:1, :])
            nc.sync.dma_start(aT[3:4, :], nlO[0:1, :])
            nc.sync.dma_start(bT[0:1, :], nhA[0:1, :])
            nc.sync.dma_start(bT[1:2, :], nlA[0:1, :])
            h2x = dpool.tile([4, 2048], F16, tag="dpk", name="h2x")
            l2x = dpool.tile([4, 2048], F16, tag="dpk", name="l2x")
            nc.scalar.mul(h2x[:], hiO[:], 2.0)
            nc.scalar.mul(l2x[:], loO[:], 2.0)
            nc.sync.dma_start(aT[4:7, :], h2x[1:4, :])
            nc.sync.dma_start(aT[7:10, :], h2x[1:4, :])
            nc.sync.dma_start(aT[10:13, :], l2x[1:4, :])
            nc.sync.dma_start(bT[4:7, :], hiA[1:4, :])
            nc.sync.dma_start(bT[7:10, :], loA[1:4, :])
            nc.sync.dma_start(bT[10:13, :], hiA[1:4, :])

            # ya (all candidates, duplicated to both partition halves) and
            # yb (own points, gather-ordered packing)
            ya_dup = gpool.tile([128, N], F32, tag="ya_dup")
            yb_p = gpool.tile([128, PL], F32, tag="yb_p")
            for dsth in range(2):
                po = 64 * dsth
                psy = pp.tile([128, HB], F32, tag="pp")
                for srch in range(2):
                    for j in range(4):
                        sl = slice(512 * j, 512 * (j + 1))
                        nc.tensor.matmul(
                            psy[po:po + 64, sl],
                            waT[bi][64 * srch:64 * srch + C, :],
                            xa_p[64 * srch:64 * srch + C, sl])
                    nc.scalar.copy(
                        ya_dup[po:po + 64, HB * srch:HB * (srch + 1)],
                        psy[po:po + 64, 0:HB])
                psb = pp.tile([128, HB], F32, tag="pp")
                for j in range(2):
                    sl = slice(512 * j, 512 * (j + 1))
                    nc.tensor.matmul(
                        psb[po:po + 64, sl],
                        wbT[bi][64 * dsth:64 * dsth + C, :],
                        xo_p[64 * dsth:64 * dsth + C, sl])
                nc.scalar.copy(
                    yb_p[po:po + 64, :].rearrange(
                        "p (g t q) -> p t g q", g=8, t=8, q=16),
                    psb[po:po + 64, 0:PL].rearrange(
                        "p (t g q) -> p t g q", t=8, g=8, q=16))

            # ---- distance + selection over own row tiles ----
            colbuf = idxpool.tile([128, 24 * NT], U16, tag="colbuf")
            posall = idxpool.tile([128, 24 * NT], U16, tag="posall")
            v24all = idxpool.tile([128, 24 * NT], F32, tag="v24all")
            for t in range(NT):
                lhs = aT[0:13, 128 * t:128 * (t + 1)]
                cand = selpool.tile([128, 128], F32, tag="cand")
                for h in range(2):
                    psd = pp.tile([128, HB], F32, tag="pp")
                    for j in range(4):
                        nc.tensor.matmul(
                            psd[:, 512 * j:512 * (j + 1)], lhs,
                            bT[0:13, HB * h + 512 * j:HB * h + 512 * (j + 1)])
                    dpk = dpool.tile([128, HB], U32, tag="dpk")
                    nc.vector.scalar_tensor_tensor(
                        dpk[:].rearrange("p (a c) -> p a c", c=CHUNK),
                        psd[:].bitcast(U32).rearrange("p (a c) -> p a c", c=CHUNK),
                        maskc[:, :],
                        iota256[:].rearrange("p (a c) -> p a c", a=1).broadcast_to([128, 8, CHUNK]),
                        ALU.bitwise_and, ALU.bitwise_or)
                    for c in range(8):
                        nc.vector.max(
                            cand[:, 64 * h + 8 * c:64 * h + 8 * (c + 1)],
                            dpk[:].bitcast(F32)[:, CHUNK * c:CHUNK * (c + 1)])
                v24 = v24all[:, 24 * t:24 * (t + 1)]
                pos = posall[:, 24 * t:24 * (t + 1)]
                c2 = selpool.tile([128, 128], F32, tag="c2")
                c3 = selpool.tile([128, 128], F32, tag="c3")
                nc.vector.max(v24[:, 0:8], cand[:])
                nc.vector.match_replace(c2[:], v24[:, 0:8], cand[:], NEG)
                nc.vector.max(v24[:, 8:16], c2[:])
                nc.vector.match_replace(c3[:], v24[:, 8:16], c2[:], NEG)
                nc.vector.max(v24[:, 16:24], c3[:])
                nc.vector.max_index(pos[:, 0:8], v24[:, 0:8], cand[:])
                nc.vector.max_index(pos[:, 8:16], v24[:, 8:16], cand[:])
                nc.vector.max_index(pos[:, 16:24], v24[:, 16:24], cand[:])
            # batched column arithmetic: col = (pos>>3)*256 + (v24.bits & 255)
            locb = idxpool.tile([128, 24 * NT], U32, tag="locb")
            ts(locb[:], v24all[:].bitcast(U32), 255, ALU.bitwise_and)
            loc16b = idxpool.tile([128, 24 * NT], U16, tag="loc16b")
            nc.vector.tensor_copy(loc16b[:], locb[:])
            ts(posall[:], posall[:], 3, ALU.logical_shift_right)
            ts(posall[:], posall[:], 8, ALU.logical_shift_left)
            nc.vector.tensor_tensor(
                colbuf[:].rearrange("p (j t) -> p t j", j=24),
                posall[:].rearrange("p (t j) -> p t j", j=24),
                loc16b[:].rearrange("p (t j) -> p t j", j=24), ALU.add)

            # ---- reformat into wrapped gather index lists ----
            # list (per own sub-half h2 of 1024 pts): position i = j*1024 + c
            # (c = gather-order column); wrapped-16: partition 64*h2+16k+(i%16),
            # free i//16 = j*64 + g2*8 + t
            wrapped = idxpool.tile([128, KNN * 64], U16, tag="wrapped")
            for h2 in range(2):
                for g2 in range(8):
                    src = colbuf[16 * g2:16 * (g2 + 1), :] \
                        .rearrange("p (j t) -> p j t", t=NT)[:, 0:KNN, 8 * h2:8 * (h2 + 1)]
                    dst = wrapped[64 * h2:64 * h2 + 16, :] \
                        .rearrange("p (j g t) -> p j g t", g=8, t=8)[:, :, g2, :]
                    nc.sync.dma_start(dst, src)
                for k in range(1, 4):
                    nc.sync.dma_start(
                        wrapped[64 * h2 + 16 * k:64 * h2 + 16 * (k + 1), :],
                        wrapped[64 * h2:64 * h2 + 16, :])

            ya3 = ya_dup[:].rearrange("p (m d) -> p m d", d=1)
            wri = wrapped[:].bitcast(I16)

            # ---- pass 1: streamed gather -> h1 chunks -> bn stats ----
            M = gpool.tile([128, PL], F32, tag="Mx")
            nc.gpsimd.memset(M[:], NEG)
            h1sum = small("h1sum", (128, KNN))
            h1sq = small("h1sq", (128, KNN))
            sscr1 = chpool.tile([128, PL], F32, tag="sscr", bufs=1, name="sscr1")
            for q in range(KNN):
                gch = chpool.tile([128, PL], F32, tag="gch", bufs=3)
                nc.gpsimd.ap_gather(
                    gch[:], ya3, wri[:, 64 * q:64 * (q + 1)],
                    channels=128, num_elems=N, d=1, num_idxs=PL)
                nc.vector.scalar_tensor_tensor(
                    gch[:], gch[:], 1.0, yb_p[:], ALU.mult, ALU.add,
                    accum_out=h1sum[:, q:q + 1])
                nc.scalar.activation(
                    sscr1[:], gch[:], AF.Square, accum_out=h1sq[:, q:q + 1])
                if not has_conv2:
                    nc.vector.tensor_tensor(M[:], M[:], gch[:], ALU.max)
            pair1 = small("mv_pair", (128, 2))
            nc.vector.tensor_reduce(pair1[:, 0:1], h1sum[:], axis=AX.X, op=ALU.add)
            nc.vector.tensor_reduce(pair1[:, 1:2], h1sq[:], axis=AX.X, op=ALU.add)
            tot1 = allreduce2(pair1)
            mv1 = mv_from_totals(tot1)
            s1, b1 = scale_bias_from_mv(mv1)

            if has_conv2:
                # pass 2: re-gather, normalize+lrelu, conv2, running max + sums
                g1s = small("g1s", (128, KNN))
                ssq = small("h2sq", (128, KNN))
                sscr = chpool.tile([128, PL], F32, tag="sscr", bufs=1)
                for j in range(KNN):
                    gch = chpool.tile([128, PL], F32, tag="gch2", bufs=3)
                    nc.gpsimd.ap_gather(
                        gch[:], ya3, wri[:, 64 * j:64 * (j + 1)],
                        channels=128, num_elems=N, d=1, num_idxs=PL)
                    nc.vector.scalar_tensor_tensor(
                        gch[:], gch[:], 1.0, yb_p[:], ALU.mult, ALU.add)
                    nc.scalar.activation(
                        gch[:], gch[:], AF.Prelu, bias=b1[:, :], scale=s1[:, :],
                        alpha=0.2, accum_out=g1s[:, j:j + 1])
                    psc = pp.tile([128, HB], F32, tag="pp")
                    for h in range(2):
                        for jj in range(2):
                            sl = slice(512 * jj, 512 * (jj + 1))
                            nc.tensor.matmul(
                                psc[64 * h:64 * h + 64, sl],
                                wcT[bi][64 * h:64 * h + 64, :],
                                gch[64 * h:64 * h + 64, sl])
                    nc.vector.tensor_tensor(M[:], M[:], psc[:, 0:PL], ALU.max)
                    nc.scalar.activation(
                        sscr[:], psc[:, 0:PL], AF.Square, accum_out=ssq[:, j:j + 1])
                pair2 = small("mv_pair", (128, 2))
                nc.vector.tensor_reduce(pair2[:, 0:1], g1s[:], axis=AX.X, op=ALU.add)
                nc.vector.tensor_reduce(pair2[:, 1:2], ssq[:], axis=AX.X, op=ALU.add)
                tot2 = allreduce2(pair2)
                # sum(h2) per channel-half = W2 @ (global sum of g)
                pss = pp.tile([128, HB], F32, tag="pp")
                nc.tensor.matmul(pss[0:64, 0:1], wcT[bi][0:64, :], tot2[0:64, 0:1])
                nc.tensor.matmul(pss[64:128, 0:1], wcT[bi][64:128, :], tot2[64:128, 0:1])
                tot2b = small("tot2b", (128, 2))
                nc.scalar.copy(tot2b[:, 0:1], pss[:, 0:1])
                nc.vector.tensor_copy(tot2b[:, 1:2], tot2[:, 1:2])
                mv2 = mv_from_totals(tot2b)
                sx, bx = scale_bias_from_mv(mv2)
            else:
                sx, bx = s1, b1

            xout_o = xown[bi + 1]
            nc.scalar.activation(
                xout_o[:].rearrange("p (t g q) -> p g t q", t=8, g=8, q=16),
                M[:], AF.Prelu, bias=bx[:, :], scale=sx[:, :], alpha=0.2)

            # AllGather own features -> all-point packed layout for next stage
            gin = cpool.tile([128, PL], F32, tag="agin")
            gout = cpool.tile([256, PL], F32, tag="agout")
            nc.gpsimd.dma_start(gin[:], xout_o[:])
            nc.gpsimd.collective_compute(
                "AllGather", ALU.bypass, replica_groups=GRPS,
                ins=[gin.opt()], outs=[gout.opt()])
            xa_next = xall[bi + 1]
            nc.gpsimd.dma_start(xa_next[0:64, 0:PL], gout[0:64, :])
            nc.gpsimd.dma_start(xa_next[0:64, PL:HB], gout[64:128, :])
            nc.gpsimd.dma_start(xa_next[64:128, 0:PL], gout[128:192, :])
            nc.gpsimd.dma_start(xa_next[64:128, PL:HB], gout[192:256, :])
            return xa_next, xout_o

        xa, xo = xall[0], xown[0]
        for bi in range(3):
            xa, xo = edge_block(bi, xa, xo, 4 if bi == 0 else 64)
        ectx.close()

        # ---------------- head (duplicated, all N points) ----------------
        x1_p, x2_p, x3_p = xall[1], xall[2], xall[3]
        hb_pool = ctx.enter_context(tc.tile_pool(name="hb_pool", bufs=1))
        hu_pool = ctx.enter_context(tc.tile_pool(name="hu_pool", bufs=3))
        xb = []
        for i, xpp in enumerate([x1_p, x2_p, x3_p]):
            t = hb_pool.tile([128, HB], BF16, tag=f"xb{i}")
            nc.scalar.copy(t[:], xpp[:])
            xb.append(t)

        # folded per-ki sums of the concat features (for the matmul sum trick)
        s64 = []
        for ki in range(3):
            sx = small(f"sx{ki}")
            nc.vector.tensor_reduce(sx[:], xb[ki][:], axis=AX.X, op=ALU.add)
            sxb = small(f"sxb{ki}", (64, 1))
            nc.sync.dma_start(sxb[:], sx[64:128, :])
            sf = small(f"sf{ki}", (64, 1), BF16)
            nc.vector.tensor_tensor(sf[:], sx[0:64, :], sxb[:], ALU.add)
            s64.append(sf)

        def stats_from_sums(ssum, ssq, count):
            """ssum/ssq [128,1] f32 over `count` -> s,b with normalized=s*x+b."""
            m = small("st2_m"); v = small("st2_v")
            ts(m[:], ssum, 1.0 / count, ALU.mult)
            ts(v[:], ssq, 1.0 / count, ALU.mult)
            m2 = small("st2_m2")
            nc.vector.tensor_tensor(m2[:], m[:], m[:], ALU.mult)
            nc.vector.tensor_tensor(v[:], v[:], m2[:], ALU.subtract)
            ts(v[:], v[:], 1e-5, ALU.add)
            s = small("st2_s"); b = small("st2_b")
            rsqrt_inplace(s[:], v[:], 128)
            nc.vector.tensor_tensor(b[:], m[:], s[:], ALU.mult)
            ts(b[:], b[:], -1.0, ALU.mult)
            return s, b

        gvecb = hb_pool.tile([128, 8], BF16, tag="gvecb")
        for g in range(8):
            psS = pp.tile([128, HB], F32, tag="pp")
            for ki in range(3):
                nc.tensor.matmul(
                    psS[:, 0:1], w6T[ki][0:64, 128 * g:128 * (g + 1)],
                    s64[ki][:], start=(ki == 0), stop=(ki == 2))
            sum6 = small("sum6")
            nc.scalar.copy(sum6[:], psS[:, 0:1])
            hq6 = small("hq6", (128, 8))
            M6 = hu_pool.tile([128, 512], F32, tag="m6", bufs=2)
            for h in range(2):
                for ci in range(4):
                    sl = slice(512 * ci, 512 * (ci + 1))
                    ps6 = pp.tile([128, HB], F32, tag="pp")
                    for ki in range(3):
                        nc.tensor.matmul(
                            ps6[:, 0:512],
                            w6T[ki][64 * h:64 * h + 64, 128 * g:128 * (g + 1)],
                            xb[ki][64 * h:64 * h + 64, sl],
                            start=(ki == 0), stop=(ki == 2))
                    cidx = 4 * h + ci
                    sq6scr = hu_pool.tile([128, 512], F32, tag="sqscr", bufs=2)
                    nc.scalar.activation(
                        sq6scr[:], ps6[:, 0:512], AF.Square,
                        accum_out=hq6[:, cidx:cidx + 1])
                    if cidx == 0:
                        nc.vector.tensor_copy(M6[:], ps6[:, 0:512])
                    else:
                        nc.vector.tensor_tensor(M6[:], M6[:], ps6[:, 0:512], ALU.max)
            sq1 = small("sq1")
            nc.vector.tensor_reduce(sq1[:], hq6[:], axis=AX.X, op=ALU.add)
            s, b = stats_from_sums(sum6[:], sq1[:], N)
            m1 = small("m1")
            nc.vector.tensor_reduce(m1[:], M6[:], axis=AX.X, op=ALU.max)
            nc.scalar.activation(
                gvecb[:, g:g + 1], m1[:], AF.Prelu, bias=b[:, :], scale=s[:, :],
                alpha=0.2)

        bias7 = hb_pool.tile([128, 4], F32, tag="bias7")
        ps7b = pp.tile([128, HB], F32, tag="pp")
        for og in range(4):
            for g in range(8):
                nc.tensor.matmul(
                    ps7b[:, og:og + 1],
                    w7gT[g][:, 128 * og:128 * (og + 1)],
                    gvecb[:, g:g + 1],
                    start=(g == 0), stop=(g == 7))
        nc.scalar.copy(bias7[:], ps7b[:, 0:4])

        h7b = []
        h7sum = []
        for og in range(4):
            psS = pp.tile([128, HB], F32, tag="pp")
            for ki in range(3):
                nc.tensor.matmul(
                    psS[:, 0:1], w7xT[ki][0:64, 128 * og:128 * (og + 1)],
                    s64[ki][:], start=(ki == 0), stop=(ki == 2))
            sum7 = small("sum7")
            nc.scalar.copy(sum7[:], psS[:, 0:1])
            nc.vector.scalar_tensor_tensor(
                sum7[:], bias7[:, og:og + 1], float(N), sum7[:], ALU.mult, ALU.add)
            u7 = hu_pool.tile([128, N], F32, tag="uh", name="u7")
            hq7 = small("hq7", (128, 8))
            for h in range(2):
                for ci in range(4):
                    sl = slice(512 * ci, 512 * (ci + 1))
                    ps7 = pp.tile([128, HB], F32, tag="pp")
                    for ki in range(3):
                        nc.tensor.matmul(
                            ps7[:, 0:512],
                            w7xT[ki][64 * h:64 * h + 64, 128 * og:128 * (og + 1)],
                            xb[ki][64 * h:64 * h + 64, sl],
                            start=(ki == 0), stop=(ki == 2))
                    usl = slice(HB * h + 512 * ci, HB * h + 512 * (ci + 1))
                    nc.scalar.activation(
                        u7[:, usl], ps7[:, 0:512],
                        AF.Identity, bias=bias7[:, og:og + 1])
                    cidx = 4 * h + ci
                    sq7scr = hu_pool.tile([128, 512], F32, tag="sqscr", bufs=2)
                    nc.vector.scalar_tensor_tensor(
                        sq7scr[:], u7[:, usl], 1.0, u7[:, usl], ALU.mult,
                        ALU.mult, accum_out=hq7[:, cidx:cidx + 1])
            sq1 = small("sq1")
            nc.vector.tensor_reduce(sq1[:], hq7[:], axis=AX.X, op=ALU.add)
            s, b = stats_from_sums(sum7[:], sq1[:], N)
            t = hb_pool.tile([128, N], BF16, tag=f"h7b{og}")
            hs = small(f"h7s{og}")
            nc.scalar.activation(
                t[:], u7[:], AF.Prelu, bias=b[:, :], scale=s[:, :], alpha=0.2,
                accum_out=hs[:])
            h7b.append(t)
            h7sum.append(hs)
        h7sum16 = small("h7sum16", (128, 4), BF16)
        for ki in range(4):
            nc.vector.tensor_copy(h7sum16[:, ki:ki + 1], h7sum[ki][:])

        h8b = []
        for og in range(2):
            psS = pp.tile([128, HB], F32, tag="pp")
            for ki in range(4):
                nc.tensor.matmul(
                    psS[:, 0:1], w8T[ki][:, 128 * og:128 * (og + 1)],
                    h7sum16[:, ki:ki + 1], start=(ki == 0), stop=(ki == 3))
            sum8 = small("sum8")
            nc.scalar.copy(sum8[:], psS[:, 0:1])
            u8 = hu_pool.tile([128, N], F32, tag="uh", name="u8")
            hq8 = small("hq8", (128, 8))
            for ci in range(8):
                sl = slice(512 * ci, 512 * (ci + 1))
                ps8 = pp.tile([128, HB], F32, tag="pp")
                for ki in range(4):
                    nc.tensor.matmul(
                        ps8[:, 0:512],
                        w8T[ki][:, 128 * og:128 * (og + 1)],
                        h7b[ki][:, sl],
                        start=(ki == 0), stop=(ki == 3))
                nc.scalar.copy(u8[:, sl], ps8[:, 0:512])
                sq8scr = hu_pool.tile([128, 512], F32, tag="sqscr", bufs=2)
                nc.vector.scalar_tensor_tensor(
                    sq8scr[:], u8[:, sl], 1.0, u8[:, sl], ALU.mult,
                    ALU.mult, accum_out=hq8[:, ci:ci + 1])
            sq1 = small("sq1")
            nc.vector.tensor_reduce(sq1[:], hq8[:], axis=AX.X, op=ALU.add)
            s, b = stats_from_sums(sum8[:], sq1[:], N)
            t = hb_pool.tile([128, N], BF16, tag=f"h8b{og}")
            nc.scalar.activation(t[:], u8[:], AF.Prelu, bias=b[:, :], scale=s[:, :], alpha=0.2)
            h8b.append(t)

        o2 = hu_pool.tile([2, N], F32, tag="uh", name="o2")
        for ci in range(8):
            sl = slice(512 * ci, 512 * (ci + 1))
            ps9 = pp.tile([128, HB], F32, tag="pp")
            for ki in range(2):
                nc.tensor.matmul(
                    ps9[0:2, 0:512],
                    w9T[ki][:], h8b[ki][:, sl],
                    start=(ki == 0), stop=(ki == 1))
            nc.scalar.copy(o2[:, sl], ps9[0:2, 0:512])

        ost = hb_pool.tile([128, 64], F32, tag="ost")
        pst = pp.tile([128, HB], F32, tag="pp")
        for t in range(32):
            nc.tensor.transpose(
                pst[:, 2 * t:2 * (t + 1)], o2[:, 128 * t:128 * (t + 1)], id2[:])
        nc.scalar.copy(ost[:], pst[:, 0:64])
        nc.sync.dma_start(
            out_d.rearrange("(t p) c -> p t c", p=128),
            ost[:].rearrange("p (t c) -> p t c", c=2))

    nc.finalize()
    return nc


def _shared_inputs(ws):
    import ml_dtypes
    w1, w2, w3, w4, w5, w6, w7, w8, w9 = ws
    f32 = np.float32
    bf16 = ml_dtypes.bfloat16
    d = {}
    for i, w in [(1, w1), (3, w3), (5, w5)]:
        C = w.shape[1] // 2
        d[f"waT{i}"] = np.ascontiguousarray(w[:, :C].T.astype(f32))
        d[f"wbT{i}"] = np.ascontiguousarray((w[:, C:] - w[:, :C]).T.astype(f32))
    d["w2T"] = np.ascontiguousarray(w2.T.astype(f32))
    d["w4T"] = np.ascontiguousarray(w4.T.astype(f32))
    w6t = w6.T.astype(bf16); w7gt = w7[:, :1024].T.astype(bf16)
    w7xt = w7[:, 1024:].T.astype(bf16); w8t = w8.T.astype(bf16)
    w9t = w9.T.astype(bf16)
    for k in range(3):
        d[f"w6T_{k}"] = np.ascontiguousarray(w6t[64 * k:64 * (k + 1)])
        d[f"w7xT_{k}"] = np.ascontiguousarray(w7xt[64 * k:64 * (k + 1)])
    for k in range(8):
        d[f"w7gT_{k}"] = np.ascontiguousarray(w7gt[128 * k:128 * (k + 1)])
    for k in range(4):
        d[f"w8T_{k}"] = np.ascontiguousarray(w8t[128 * k:128 * (k + 1)])
    for k in range(2):
        d[f"w9T_{k}"] = np.ascontiguousarray(w9t[128 * k:128 * (k + 1)])
    d["iota256"] = np.broadcast_to(
        np.arange(CHUNK, dtype=np.uint32)[None, :], (128, CHUNK)).copy()
    d["maskc"] = np.full((128, 1), 0xFFFFFF00, dtype=np.uint32)
    d["magic"] = np.full((128, 1), 0x5F3759DF, dtype=np.uint32)
    d["id2"] = np.eye(2, dtype=f32)
    return d


def _run(inputs, want_debug=False):
    from concourse.bass_utils import run_bass_kernel_spmd

    if "nc" not in _CACHE:
        _CACHE["nc"] = _build_program()
    nc = _CACHE["nc"]

    x = np.asarray(inputs["x"], dtype=np.float32)
    ws = [np.asarray(inputs[f"w{i}"], dtype=np.float32) for i in range(1, 10)]
    shared = _shared_inputs(ws)
    in_maps = []
    for c in range(8):
        s, h = c // 2, c % 2
        xt = np.ascontiguousarray(x[s].T.astype(np.float32))
        m = dict(shared)
        m["xt"] = xt
        m["xo"] = np.ascontiguousarray(xt[:, 2048 * h:2048 * (h + 1)])
        in_maps.append(m)
    res = run_bass_kernel_spmd(nc, in_maps, list(range(8)))
    out = np.stack([res.results[2 * s]["out"] for s in range(4)])
    if want_debug:
        return out, [res.results[c] for c in range(8)]
    return out


def kernel(**inputs):
    return _run(inputs)


# revision 25
# speedup vs baseline: 1.5215x; 1.2961x over previous
"""DGCNN semantic-segmentation kernel for 8x Trainium2 NeuronCores.

Strategy: 2 cores per sample. Core c handles sample c//2, point-half c%2
(2048 of 4096 points). Per block each core computes kNN rows / gather /
EdgeConv for its own points only; pair collectives supply the global parts:
an AllReduce of instance-norm partial sums and an AllGather of the block
output (the next block needs all candidate points). The head (w6..w9) is
cheap and runs duplicated on both cores over all N points; the host takes
even cores' outputs.

Per-core pipeline per block:
  kNN (fp16 hi/lo split K=13 distance matmul, own 2048 rows x all 4096
  cols) -> top-20 selection (pack 8-bit chunk-local index into low mantissa
  bits, top-8 per 256-chunk via DVE max8, refine, recover columns) ->
  streamed gather via GPSIMD ap_gather -> folded 1x1 conv + instance-norm
  (pair-AllReduced stats) + leaky-relu -> second conv streamed with running
  max over k -> prelu -> AllGather of the new features.

Own-point packed layout: [128, 1024] - partition p<64 holds channel p for
own-local points n<1024, partition 64+p for n>=1024. All-point packed
layout: [128, 2048] likewise split at 2048.
"""

import sys

if "/opt/trn_rl_repo" not in sys.path:
    sys.path.insert(0, "/opt/trn_rl_repo")

import numpy as np

N = 4096
NT = 16          # row tiles of 128 for the distance/selection loop (own pts)
HB = 2048        # free size of all-point packed tiles
PL = 1024        # free size of own-point packed tiles
KNN = 20
NK = KNN * HB    # global per-channel-half element count for in2d stats
CHUNK = 256
NEG = -3.0e38
GRPS = [[0, 1], [2, 3], [4, 5], [6, 7]]

_CACHE = {}


def _build_program():
    import concourse.bacc as bacc
    import concourse.tile as tile
    from concourse import mybir
    from contextlib import ExitStack

    F32 = mybir.dt.float32
    F16 = mybir.dt.float16
    BF16 = mybir.dt.bfloat16
    U32 = mybir.dt.uint32
    U16 = mybir.dt.uint16
    I16 = mybir.dt.int16
    AF = mybir.ActivationFunctionType
    ALU = mybir.AluOpType
    AX = mybir.AxisListType

    nc = bacc.Bacc("TRN2", target_bir_lowering=False, debug=False, num_devices=8)

    def din(name, shape, dt=F32):
        return nc.dram_tensor(name, shape, dt, kind="ExternalInput").ap()

    xt_d = din("xt", [4, N])
    xo_d = din("xo", [4, 2048])
    waT_d = [din("waT1", [4, 64]), din("waT3", [64, 64]), din("waT5", [64, 64])]
    wbT_d = [din("wbT1", [4, 64]), din("wbT3", [64, 64]), din("wbT5", [64, 64])]
    wcT_d = [din("w2T", [64, 64]), din("w4T", [64, 64])]
    w6T_d = [din(f"w6T_{k}", [64, 1024], BF16) for k in range(3)]
    w7gT_d = [din(f"w7gT_{k}", [128, 512], BF16) for k in range(8)]
    w7xT_d = [din(f"w7xT_{k}", [64, 512], BF16) for k in range(3)]
    w8T_d = [din(f"w8T_{k}", [128, 256], BF16) for k in range(4)]
    w9T_d = [din(f"w9T_{k}", [128, 2], BF16) for k in range(2)]
    iota256_d = din("iota256", [128, CHUNK], U32)
    maskc_d = din("maskc", [128, 1], U32)
    magic_d = din("magic", [128, 1], U32)
    id2_d = din("id2", [2, 2])

    out_d = nc.dram_tensor("out", [N, 2], F32, kind="ExternalOutput").ap()

    with tile.TileContext(nc) as tc, ExitStack() as ctx:
        wpool = ctx.enter_context(tc.tile_pool(name="wpool", bufs=1))
        xpool = ctx.enter_context(tc.tile_pool(name="xpool", bufs=1))
        stpool = ctx.enter_context(tc.tile_pool(name="stpool", bufs=1))
        pp = ctx.enter_context(tc.tile_pool(name="pp", bufs=2, space="PSUM"))
        cpool = ctx.enter_context(tc.tile_pool(name="cdram", bufs=2, space="DRAM"))
        ectx = ExitStack()
        abpool = ectx.enter_context(tc.tile_pool(name="abpool", bufs=1))
        gpool = ectx.enter_context(tc.tile_pool(name="gpool", bufs=1))
        dpool = ectx.enter_context(tc.tile_pool(name="dpool", bufs=2))
        selpool = ectx.enter_context(tc.tile_pool(name="selpool", bufs=4))
        idxpool = ectx.enter_context(tc.tile_pool(name="idxpool", bufs=1))
        chpool = ectx.enter_context(tc.tile_pool(name="chpool", bufs=1))

        def load(pool, ap_d, shape, dt=F32, dup64=False):
            rows = shape[0]
            tshape = [128, shape[1]] if dup64 else shape
            t = pool.tile(tshape, dt, tag=ap_d.tensor.name, name=ap_d.tensor.name + "_sb")
            nc.sync.dma_start(t[0:rows, :], ap_d)
            if dup64:
                nc.sync.dma_start(t[64:64 + rows, :], ap_d)
            return t

        waT = [load(wpool, waT_d[0], [4, 64], dup64=True),
               load(wpool, waT_d[1], [64, 64], dup64=True),
               load(wpool, waT_d[2], [64, 64], dup64=True)]
        wbT = [load(wpool, wbT_d[0], [4, 64], dup64=True),
               load(wpool, wbT_d[1], [64, 64], dup64=True),
               load(wpool, wbT_d[2], [64, 64], dup64=True)]
        wcT = [load(wpool, wcT_d[0], [64, 64], dup64=True),
               load(wpool, wcT_d[1], [64, 64], dup64=True), None]
        w6T = [load(wpool, a, [64, 1024], BF16, dup64=True) for a in w6T_d]
        w7gT = [load(wpool, a, [128, 512], BF16) for a in w7gT_d]
        w7xT = [load(wpool, a, [64, 512], BF16, dup64=True) for a in w7xT_d]
        w8T = [load(wpool, a, [128, 256], BF16) for a in w8T_d]
        w9T = [load(wpool, a, [128, 2], BF16) for a in w9T_d]
        iota256 = load(wpool, iota256_d, [128, CHUNK], U32)
        maskc = load(wpool, maskc_d, [128, 1], U32)
        magic = load(wpool, magic_d, [128, 1], U32)
        id2 = load(wpool, id2_d, [2, 2])

        xall = [xpool.tile([128, HB], F32, tag=f"xa{i}", name=f"xa{i}")
                for i in range(4)]
        xown = [xpool.tile([128, PL], F32, tag=f"xw{i}", name=f"xw{i}")
                for i in range(4)]
        nc.sync.dma_start(xall[0][0:4, :], xt_d[:, 0:HB])
        nc.sync.dma_start(xall[0][64:68, :], xt_d[:, HB:N])
        nc.sync.dma_start(xown[0][0:4, :], xo_d[:, 0:PL])
        nc.sync.dma_start(xown[0][64:68, :], xo_d[:, PL:2048])

        def small(tag, shape=(128, 1), dt=F32):
            return stpool.tile(list(shape), dt, tag=tag, name=tag)

        def ts(out, in0, s1, op0, s2=None, op1=None):
            if op1 is None:
                nc.vector.tensor_scalar(out, in0, s1, None, op0)
            else:
                nc.vector.tensor_scalar(out, in0, s1, s2, op0, op1)

        def rsqrt_inplace(y, t_in, rows):
            b = small("rs_b", (rows, 1), U32)
            ts(b[:], t_in.bitcast(U32), 1, ALU.logical_shift_right)
            nc.vector.tensor_tensor(y.bitcast(U32), magic[0:rows, :], b[:], ALU.subtract)
            for _ in range(2):
                u = small("rs_u", (rows, 1))
                nc.vector.tensor_tensor(u[:], y, y, ALU.mult)
                nc.vector.tensor_tensor(u[:], u[:], t_in, ALU.mult)
                ts(u[:], u[:], -0.5, ALU.mult, 1.5, ALU.add)
                nc.vector.tensor_tensor(y, y, u[:], ALU.mult)

        def allreduce2(pair):
            """pair [128,2] f32 local partials -> [128,2] summed over the
            2-core pair. AllGather + local fold (AllGather is ~2x cheaper
            than AllReduce in latency)."""
            din_t = cpool.tile([128, 2], F32, tag="arin")
            dout_t = cpool.tile([256, 2], F32, tag="arout")
            nc.gpsimd.dma_start(din_t[:], pair[:])
            nc.gpsimd.collective_compute(
                "AllGather", ALU.bypass, replica_groups=GRPS,
                ins=[din_t.opt()], outs=[dout_t.opt()])
            both = small("ar_both", (128, 4))
            nc.gpsimd.dma_start(both[:, 0:2], dout_t[0:128, :])
            nc.gpsimd.dma_start(both[:, 2:4], dout_t[128:256, :])
            res = small("ar_res", (128, 2))
            nc.vector.tensor_tensor(res[:], both[:, 0:2], both[:, 2:4], ALU.add)
            return res

        def scale_bias_from_mv(mv):
            """mv [128,2] per-partition (mean, var); rows p/p+64 are halves of one
            channel. Returns s128, b128 [128,1] with normalized = s*x + b."""
            mvb = small("st_mvb", (64, 2))
            nc.sync.dma_start(mvb[:], mv[64:128, :])
            m = small("st_m", (64, 1)); v = small("st_v", (64, 1))
            dm = small("st_dm", (64, 1))
            nc.vector.tensor_tensor(m[:], mv[0:64, 0:1], mvb[:, 0:1], ALU.add)
            ts(m[:], m[:], 0.5, ALU.mult)
            nc.vector.tensor_tensor(v[:], mv[0:64, 1:2], mvb[:, 1:2], ALU.add)
            nc.vector.tensor_tensor(dm[:], mv[0:64, 0:1], mvb[:, 0:1], ALU.subtract)
            nc.vector.tensor_tensor(dm[:], dm[:], dm[:], ALU.mult)
            ts(v[:], v[:], 0.5, ALU.mult)
            ts(dm[:], dm[:], 0.25, ALU.mult)
            nc.vector.tensor_tensor(v[:], v[:], dm[:], ALU.add)
            ts(v[:], v[:], 1e-5, ALU.add)
            s = small("st_s", (64, 1))
            rsqrt_inplace(s[:], v[:], 64)
            bb = small("st_bb", (64, 1))
            nc.vector.tensor_tensor(bb[:], m[:], s[:], ALU.mult)
            ts(bb[:], bb[:], -1.0, ALU.mult)
            s128 = small("st_s128"); b128 = small("st_b128")
            nc.vector.tensor_copy(s128[0:64, :], s[:])
            nc.vector.tensor_copy(b128[0:64, :], bb[:])
            nc.sync.dma_start(s128[64:128, :], s[:])
            nc.sync.dma_start(b128[64:128, :], bb[:])
            return s128, b128

        def mv_from_totals(tot):
            """tot [128,2] global (sum, sqsum) -> mv [128,2] (mean, var)."""
            mv = small("sm_mv", (128, 2))
            ts(mv[:, 0:1], tot[:, 0:1], 1.0 / NK, ALU.mult)
            ts(mv[:, 1:2], tot[:, 1:2], 1.0 / NK, ALU.mult)
            m2 = small("sm_m2")
            nc.vector.tensor_tensor(m2[:], mv[:, 0:1], mv[:, 0:1], ALU.mult)
            nc.vector.tensor_tensor(mv[:, 1:2], mv[:, 1:2], m2[:], ALU.subtract)
            return mv

        # ---------------- EdgeConv block ----------------
        def edge_block(bi, xa_p, xo_p, C):
            has_conv2 = bi < 2
            ones3 = abpool.tile([128, 1], F32, tag="ones3")
            nc.gpsimd.memset(ones3[:], 1.0)

            # -- distance operand rows for own points (aT) first: xo_p is
            # available before the previous block's AllGather completes, so
            # this work hides the collective latency --
            sqo = dpool.tile([128, PL], F32, tag="dpk", name="sqo")
            nc.scalar.activation(sqo[0:3, :], xo_p[0:3, :], AF.Square)
            nc.scalar.activation(sqo[64:67, :], xo_p[64:67, :], AF.Square)
            src4o = abpool.tile([4, 2048], F32, tag="src4o", name="src4o")
            for h in range(2):
                psx = pp.tile([128, HB], F32, tag="pp")
                for j in range(2):
                    nc.tensor.matmul(
                        psx[0:1, 512 * j:512 * (j + 1)],
                        ones3[64 * h:64 * h + 3, 0:1],
                        sqo[64 * h:64 * h + 3, 512 * j:512 * (j + 1)])
                nc.scalar.copy(src4o[0:1, PL * h:PL * (h + 1)], psx[0:1, 0:PL])
                nc.sync.dma_start(src4o[1:4, PL * h:PL * (h + 1)], xo_p[64 * h:64 * h + 3, :])
            hiO = gpool.tile([4, 2048], F16, tag="M", name="hiO")
            loO = gpool.tile([4, 2048], F16, tag="yb_p", name="loO")
            nc.scalar.copy(hiO[:], src4o[:])
            nc.vector.scalar_tensor_tensor(
                loO[:], hiO[:], -1.0, src4o[:], ALU.mult, ALU.add)
            nhO = dpool.tile([4, 2048], F16, tag="dpk", name="nhO")
            nlO = dpool.tile([4, 2048], F16, tag="dpk", name="nlO")
            nc.scalar.mul(nhO[:], hiO[:], -1.0)
            nc.scalar.mul(nlO[:], loO[:], -1.0)

            # aT rows (own): [1, 1, -xxh, -xxl, 2ph(3), 2ph(3), 2pl(3)]
            # bT rows (all): [-xxh, -xxl, 1, 1, ph(3), pl(3), ph(3)]
            aT = abpool.tile([16, 2048], F16, tag="aT")
            bT = abpool.tile([16, N], F16, tag="bT")
            onesb = abpool.tile([2, N], F16, tag="onesb")
            nc.gpsimd.memset(onesb[:], 1.0)
            nc.sync.dma_start(bT[2:4, :], onesb[:])
            nc.sync.dma_start(aT[0:2, :], onesb[:, 0:2048])
            nc.sync.dma_start(aT[2:3, :], nhO[0:1, :])
            nc.sync.dma_start(aT[3:4, :], nlO[0:1, :])
            h2x = dpool.tile([4, 2048], F16, tag="dpk", name="h2x")
            l2x = dpool.tile([4, 2048], F16, tag="dpk", name="l2x")
            nc.scalar.mul(h2x[:], hiO[:], 2.0)
            nc.scalar.mul(l2x[:], loO[:], 2.0)
            nc.sync.dma_start(aT[4:7, :], h2x[1:4, :])
            nc.sync.dma_start(aT[7:10, :], h2x[1:4, :])
            nc.sync.dma_start(aT[10:13, :], l2x[1:4, :])

            # -- distance operand rows for all candidates (bT): needs xa_p,
            # i.e. the previous block's AllGather --
            sqa = dpool.tile([128, HB], F32, tag="dpk", name="sqa")
            nc.scalar.activation(sqa[0:3, :], xa_p[0:3, :], AF.Square)
            nc.scalar.activation(sqa[64:67, :], xa_p[64:67, :], AF.Square)
            src4a = gpool.tile([4, N], F32, tag="ya_dup", name="src4a")
            for h in range(2):
                psx = pp.tile([128, HB], F32, tag="pp")
                for j in range(4):
                    nc.tensor.matmul(
                        psx[0:1, 512 * j:512 * (j + 1)],
                        ones3[64 * h:64 * h + 3, 0:1],
                        sqa[64 * h:64 * h + 3, 512 * j:512 * (j + 1)])
                nc.scalar.copy(src4a[0:1, HB * h:HB * (h + 1)], psx[0:1, 0:HB])
                nc.sync.dma_start(src4a[1:4, HB * h:HB * (h + 1)], xa_p[64 * h:64 * h + 3, :])
            hiA = gpool.tile([4, N], F16, tag="hiA", name="hiA")
            loA = gpool.tile([4, N], F16, tag="loA", name="loA")
            nc.scalar.copy(hiA[:], src4a[:])
            nc.vector.scalar_tensor_tensor(
                loA[:], hiA[:], -1.0, src4a[:], ALU.mult, ALU.add)
            nhA = dpool.tile([4, N], F16, tag="dpk", name="nhA")
            nlA = dpool.tile([4, N], F16, tag="dpk", name="nlA")
            nc.scalar.mul(nhA[:], hiA[:], -1.0)
            nc.scalar.mul(nlA[:], loA[:], -1.0)
            nc.sync.dma_start(bT[0:1, :], nhA[0:1, :])
            nc.sync.dma_start(bT[1:2, :], nlA[0:1, :])
            nc.sync.dma_start(bT[4:7, :], hiA[1:4, :])
            nc.sync.dma_start(bT[7:10, :], loA[1:4, :])
            nc.sync.dma_start(bT[10:13, :], hiA[1:4, :])

            # ya (all candidates, duplicated to both partition halves) and
            # yb (own points, gather-ordered packing)
            ya_dup = gpool.tile([128, N], F32, tag="ya_dup")
            yb_p = gpool.tile([128, PL], F32, tag="yb_p")
            for dsth in range(2):
                po = 64 * dsth
                psy = pp.tile([128, HB], F32, tag="pp")
                for srch in range(2):
                    for j in range(4):
                        sl = slice(512 * j, 512 * (j + 1))
                        nc.tensor.matmul(
                            psy[po:po + 64, sl],
                            waT[bi][64 * srch:64 * srch + C, :],
                            xa_p[64 * srch:64 * srch + C, sl])
                    nc.scalar.copy(
                        ya_dup[po:po + 64, HB * srch:HB * (srch + 1)],
                        psy[po:po + 64, 0:HB])
                psb = pp.tile([128, HB], F32, tag="pp")
                for j in range(2):
                    sl = slice(512 * j, 512 * (j + 1))
                    nc.tensor.matmul(
                        psb[po:po + 64, sl],
                        wbT[bi][64 * dsth:64 * dsth + C, :],
                        xo_p[64 * dsth:64 * dsth + C, sl])
                nc.scalar.copy(
                    yb_p[po:po + 64, :].rearrange(
                        "p (g t q) -> p t g q", g=8, t=8, q=16),
                    psb[po:po + 64, 0:PL].rearrange(
                        "p (t g q) -> p t g q", t=8, g=8, q=16))

            # ---- distance + selection over own row tiles ----
            colbuf = idxpool.tile([128, 24 * NT], U16, tag="colbuf")
            posall = idxpool.tile([128, 24 * NT], U16, tag="posall")
            v24all = idxpool.tile([128, 24 * NT], F32, tag="v24all")
            for t in range(NT):
                lhs = aT[0:13, 128 * t:128 * (t + 1)]
                cand = selpool.tile([128, 128], F32, tag="cand")
                for h in range(2):
                    psd = pp.tile([128, HB], F32, tag="pp")
                    for j in range(4):
                        nc.tensor.matmul(
                            psd[:, 512 * j:512 * (j + 1)], lhs,
                            bT[0:13, HB * h + 512 * j:HB * h + 512 * (j + 1)])
                    dpk = dpool.tile([128, HB], U32, tag="dpk")
                    nc.gpsimd.scalar_tensor_tensor(
                        dpk[:].rearrange("p (a c) -> p a c", c=CHUNK),
                        psd[:].bitcast(U32).rearrange("p (a c) -> p a c", c=CHUNK),
                        maskc[:, :],
                        iota256[:].rearrange("p (a c) -> p a c", a=1).broadcast_to([128, 8, CHUNK]),
                        ALU.bitwise_and, ALU.bitwise_or)
                    for c in range(8):
                        nc.vector.max(
                            cand[:, 64 * h + 8 * c:64 * h + 8 * (c + 1)],
                            dpk[:].bitcast(F32)[:, CHUNK * c:CHUNK * (c + 1)])
                v24 = v24all[:, 24 * t:24 * (t + 1)]
                pos = posall[:, 24 * t:24 * (t + 1)]
                c2 = selpool.tile([128, 128], F32, tag="c2")
                c3 = selpool.tile([128, 128], F32, tag="c3")
                nc.vector.max(v24[:, 0:8], cand[:])
                nc.vector.match_replace(c2[:], v24[:, 0:8], cand[:], NEG)
                nc.vector.max(v24[:, 8:16], c2[:])
                nc.vector.match_replace(c3[:], v24[:, 8:16], c2[:], NEG)
                nc.vector.max(v24[:, 16:24], c3[:])
                nc.vector.max_index(pos[:, 0:8], v24[:, 0:8], cand[:])
                nc.vector.max_index(pos[:, 8:16], v24[:, 8:16], cand[:])
                nc.vector.max_index(pos[:, 16:24], v24[:, 16:24], cand[:])
            # batched column arithmetic: col = (pos>>3)*256 + (v24.bits & 255)
            locb = idxpool.tile([128, 24 * NT], U32, tag="locb")
            ts(locb[:], v24all[:].bitcast(U32), 255, ALU.bitwise_and)
            loc16b = idxpool.tile([128, 24 * NT], U16, tag="loc16b")
            nc.vector.tensor_copy(loc16b[:], locb[:])
            ts(posall[:], posall[:], 3, ALU.logical_shift_right)
            ts(posall[:], posall[:], 8, ALU.logical_shift_left)
            nc.vector.tensor_tensor(
                colbuf[:].rearrange("p (j t) -> p t j", j=24),
                posall[:].rearrange("p (t j) -> p t j", j=24),
                loc16b[:].rearrange("p (t j) -> p t j", j=24), ALU.add)

            # ---- reformat into wrapped gather index lists ----
            # list (per own sub-half h2 of 1024 pts): position i = j*1024 + c
            # (c = gather-order column); wrapped-16: partition 64*h2+16k+(i%16),
            # free i//16 = j*64 + g2*8 + t
            wrapped = idxpool.tile([128, KNN * 64], U16, tag="wrapped")
            for h2 in range(2):
                for g2 in range(8):
                    src = colbuf[16 * g2:16 * (g2 + 1), :] \
                        .rearrange("p (j t) -> p j t", t=NT)[:, 0:KNN, 8 * h2:8 * (h2 + 1)]
                    dst = wrapped[64 * h2:64 * h2 + 16, :] \
                        .rearrange("p (j g t) -> p j g t", g=8, t=8)[:, :, g2, :]
                    nc.sync.dma_start(dst, src)
                for k in range(1, 4):
                    nc.sync.dma_start(
                        wrapped[64 * h2 + 16 * k:64 * h2 + 16 * (k + 1), :],
                        wrapped[64 * h2:64 * h2 + 16, :])

            ya3 = ya_dup[:].rearrange("p (m d) -> p m d", d=1)
            wri = wrapped[:].bitcast(I16)

            # ---- pass 1: batched gathers (4 chunks/instr) -> bn stats ----
            BW = 4
            NBT = KNN // BW
            M = gpool.tile([128, PL], F32, tag="Mx")
            nc.gpsimd.memset(M[:], NEG)
            h1sum = small("h1sum", (128, NBT))
            h1sq = small("h1sq", (128, NBT))
            ybb = yb_p[:].rearrange("p (a f) -> p a f", a=1).broadcast_to([128, BW, PL])
            for q in range(NBT):
                gch = chpool.tile([128, BW * PL], F32, tag="gch", bufs=2)
                nc.gpsimd.ap_gather(
                    gch[:], ya3, wri[:, 64 * BW * q:64 * BW * (q + 1)],
                    channels=128, num_elems=N, d=1, num_idxs=BW * PL)
                nc.vector.scalar_tensor_tensor(
                    gch[:].rearrange("p (j f) -> p j f", j=BW),
                    gch[:].rearrange("p (j f) -> p j f", j=BW),
                    1.0, ybb, ALU.mult, ALU.add,
                    accum_out=h1sum[:, q:q + 1])
                if not has_conv2:
                    for jj in range(BW):
                        nc.vector.tensor_tensor(
                            M[:], M[:], gch[:, PL * jj:PL * (jj + 1)], ALU.max)
                nc.scalar.activation(
                    gch[:], gch[:], AF.Square, accum_out=h1sq[:, q:q + 1])
            pair1 = small("mv_pair", (128, 2))
            nc.vector.tensor_reduce(pair1[:, 0:1], h1sum[:], axis=AX.X, op=ALU.add)
            nc.vector.tensor_reduce(pair1[:, 1:2], h1sq[:], axis=AX.X, op=ALU.add)
            tot1 = allreduce2(pair1)
            mv1 = mv_from_totals(tot1)
            s1, b1 = scale_bias_from_mv(mv1)

            if has_conv2:
                # pass 2: re-gather (batched), normalize+lrelu, conv2,
                # running max + sums
                g1s = small("g1s", (128, NBT))
                ssq = small("h2sq", (128, 2 * NBT))
                for q in range(NBT):
                    gch = chpool.tile([128, BW * PL], F32, tag="gch", bufs=2)
                    nc.gpsimd.ap_gather(
                        gch[:], ya3, wri[:, 64 * BW * q:64 * BW * (q + 1)],
                        channels=128, num_elems=N, d=1, num_idxs=BW * PL)
                    nc.vector.scalar_tensor_tensor(
                        gch[:].rearrange("p (j f) -> p j f", j=BW),
                        gch[:].rearrange("p (j f) -> p j f", j=BW),
                        1.0, ybb, ALU.mult, ALU.add)
                    nc.scalar.activation(
                        gch[:], gch[:], AF.Prelu, bias=b1[:, :], scale=s1[:, :],
                        alpha=0.2, accum_out=g1s[:, q:q + 1])
                    for hp in range(2):
                        psc = pp.tile([128, HB], F32, tag="pp")
                        for h in range(2):
                            for jj in range(4):
                                sl = slice(512 * jj, 512 * (jj + 1))
                                nc.tensor.matmul(
                                    psc[64 * h:64 * h + 64, sl],
                                    wcT[bi][64 * h:64 * h + 64, :],
                                    gch[64 * h:64 * h + 64, 2048 * hp + 512 * jj:
                                        2048 * hp + 512 * (jj + 1)])
                        nc.vector.tensor_tensor(M[:], M[:], psc[:, 0:PL], ALU.max)
                        nc.vector.tensor_tensor(M[:], M[:], psc[:, PL:HB], ALU.max)
                        nc.scalar.activation(
                            gch[:, 2048 * hp:2048 * (hp + 1)], psc[:, 0:HB],
                            AF.Square, accum_out=ssq[:, 2 * q + hp:2 * q + hp + 1])
                pair2 = small("mv_pair", (128, 2))
                nc.vector.tensor_reduce(pair2[:, 0:1], g1s[:], axis=AX.X, op=ALU.add)
                nc.vector.tensor_reduce(pair2[:, 1:2], ssq[:], axis=AX.X, op=ALU.add)
                tot2 = allreduce2(pair2)
                # sum(h2) per channel-half = W2 @ (global sum of g)
                pss = pp.tile([128, HB], F32, tag="pp")
                nc.tensor.matmul(pss[0:64, 0:1], wcT[bi][0:64, :], tot2[0:64, 0:1])
                nc.tensor.matmul(pss[64:128, 0:1], wcT[bi][64:128, :], tot2[64:128, 0:1])
                tot2b = small("tot2b", (128, 2))
                nc.scalar.copy(tot2b[:, 0:1], pss[:, 0:1])
                nc.vector.tensor_copy(tot2b[:, 1:2], tot2[:, 1:2])
                mv2 = mv_from_totals(tot2b)
                sx, bx = scale_bias_from_mv(mv2)
            else:
                sx, bx = s1, b1

            xout_o = xown[bi + 1]
            nc.scalar.activation(
                xout_o[:].rearrange("p (t g q) -> p g t q", t=8, g=8, q=16),
                M[:], AF.Prelu, bias=bx[:, :], scale=sx[:, :], alpha=0.2)

            # AllGather own features -> all-point packed layout for next stage
            gin = cpool.tile([128, PL], F32, tag="agin")
            gout = cpool.tile([256, PL], F32, tag="agout")
            nc.gpsimd.dma_start(gin[:], xout_o[:])
            nc.gpsimd.collective_compute(
                "AllGather", ALU.bypass, replica_groups=GRPS,
                ins=[gin.opt()], outs=[gout.opt()])
            xa_next = xall[bi + 1]
            nc.gpsimd.dma_start(xa_next[0:64, 0:PL], gout[0:64, :])
            nc.gpsimd.dma_start(xa_next[0:64, PL:HB], gout[64:128, :])
            nc.gpsimd.dma_start(xa_next[64:128, 0:PL], gout[128:192, :])
            nc.gpsimd.dma_start(xa_next[64:128, PL:HB], gout[192:256, :])
            return xa_next, xout_o

        xa, xo = xall[0], xown[0]
        for bi in range(3):
            xa, xo = edge_block(bi, xa, xo, 4 if bi == 0 else 64)
        ectx.close()

        # ---------------- head (duplicated, all N points) ----------------
        x1_p, x2_p, x3_p = xall[1], xall[2], xall[3]
        hb_pool = ctx.enter_context(tc.tile_pool(name="hb_pool", bufs=1))
        hu_pool = ctx.enter_context(tc.tile_pool(name="hu_pool", bufs=3))
        xb = []
        for i, xpp in enumerate([x1_p, x2_p, x3_p]):
            t = hb_pool.tile([128, HB], BF16, tag=f"xb{i}")
            nc.scalar.copy(t[:], xpp[:])
            xb.append(t)

        # folded per-ki sums of the concat features (for the matmul sum trick)
        s64 = []
        for ki in range(3):
            sx = small(f"sx{ki}")
            nc.vector.tensor_reduce(sx[:], xb[ki][:], axis=AX.X, op=ALU.add)
            sxb = small(f"sxb{ki}", (64, 1))
            nc.sync.dma_start(sxb[:], sx[64:128, :])
            sf = small(f"sf{ki}", (64, 1), BF16)
            nc.vector.tensor_tensor(sf[:], sx[0:64, :], sxb[:], ALU.add)
            s64.append(sf)

        def stats_from_sums(ssum, ssq, count):
            """ssum/ssq [128,1] f32 over `count` -> s,b with normalized=s*x+b."""
            m = small("st2_m"); v = small("st2_v")
            ts(m[:], ssum, 1.0 / count, ALU.mult)
            ts(v[:], ssq, 1.0 / count, ALU.mult)
            m2 = small("st2_m2")
            nc.vector.tensor_tensor(m2[:], m[:], m[:], ALU.mult)
            nc.vector.tensor_tensor(v[:], v[:], m2[:], ALU.subtract)
            ts(v[:], v[:], 1e-5, ALU.add)
            s = small("st2_s"); b = small("st2_b")
            rsqrt_inplace(s[:], v[:], 128)
            nc.vector.tensor_tensor(b[:], m[:], s[:], ALU.mult)
            ts(b[:], b[:], -1.0, ALU.mult)
            return s, b

        gvecb = hb_pool.tile([128, 8], BF16, tag="gvecb")
        for g in range(8):
            psS = pp.tile([128, HB], F32, tag="pp")
            for ki in range(3):
                nc.tensor.matmul(
                    psS[:, 0:1], w6T[ki][0:64, 128 * g:128 * (g + 1)],
                    s64[ki][:], start=(ki == 0), stop=(ki == 2))
            sum6 = small("sum6")
            nc.scalar.copy(sum6[:], psS[:, 0:1])
            hq6 = small("hq6", (128, 8))
            M6 = hu_pool.tile([128, 512], F32, tag="m6", bufs=2)
            for h in range(2):
                for ci in range(4):
                    sl = slice(512 * ci, 512 * (ci + 1))
                    ps6 = pp.tile([128, HB], F32, tag="pp")
                    for ki in range(3):
                        nc.tensor.matmul(
                            ps6[:, 0:512],
                            w6T[ki][64 * h:64 * h + 64, 128 * g:128 * (g + 1)],
                            xb[ki][64 * h:64 * h + 64, sl],
                            start=(ki == 0), stop=(ki == 2))
                    cidx = 4 * h + ci
                    sq6scr = hu_pool.tile([128, 512], F32, tag="sqscr", bufs=2)
                    nc.scalar.activation(
                        sq6scr[:], ps6[:, 0:512], AF.Square,
                        accum_out=hq6[:, cidx:cidx + 1])
                    if cidx == 0:
                        nc.vector.tensor_copy(M6[:], ps6[:, 0:512])
                    else:
                        nc.vector.tensor_tensor(M6[:], M6[:], ps6[:, 0:512], ALU.max)
            sq1 = small("sq1")
            nc.vector.tensor_reduce(sq1[:], hq6[:], axis=AX.X, op=ALU.add)
            s, b = stats_from_sums(sum6[:], sq1[:], N)
            m1 = small("m1")
            nc.vector.tensor_reduce(m1[:], M6[:], axis=AX.X, op=ALU.max)
            nc.scalar.activation(
                gvecb[:, g:g + 1], m1[:], AF.Prelu, bias=b[:, :], scale=s[:, :],
                alpha=0.2)

        bias7 = hb_pool.tile([128, 4], F32, tag="bias7")
        ps7b = pp.tile([128, HB], F32, tag="pp")
        for og in range(4):
            for g in range(8):
                nc.tensor.matmul(
                    ps7b[:, og:og + 1],
                    w7gT[g][:, 128 * og:128 * (og + 1)],
                    gvecb[:, g:g + 1],
                    start=(g == 0), stop=(g == 7))
        nc.scalar.copy(bias7[:], ps7b[:, 0:4])

        h7b = []
        h7sum = []
        for og in range(4):
            psS = pp.tile([128, HB], F32, tag="pp")
            for ki in range(3):
                nc.tensor.matmul(
                    psS[:, 0:1], w7xT[ki][0:64, 128 * og:128 * (og + 1)],
                    s64[ki][:], start=(ki == 0), stop=(ki == 2))
            sum7 = small("sum7")
            nc.scalar.copy(sum7[:], psS[:, 0:1])
            nc.vector.scalar_tensor_tensor(
                sum7[:], bias7[:, og:og + 1], float(N), sum7[:], ALU.mult, ALU.add)
            u7 = hu_pool.tile([128, N], F32, tag="uh", name="u7")
            hq7 = small("hq7", (128, 8))
            for h in range(2):
                for ci in range(4):
                    sl = slice(512 * ci, 512 * (ci + 1))
                    ps7 = pp.tile([128, HB], F32, tag="pp")
                    for ki in range(3):
                        nc.tensor.matmul(
                            ps7[:, 0:512],
                            w7xT[ki][64 * h:64 * h + 64, 128 * og:128 * (og + 1)],
                            xb[ki][64 * h:64 * h + 64, sl],
                            start=(ki == 0), stop=(ki == 2))
                    usl = slice(HB * h + 512 * ci, HB * h + 512 * (ci + 1))
                    nc.scalar.activation(
                        u7[:, usl], ps7[:, 0:512],
                        AF.Identity, bias=bias7[:, og:og + 1])
                    cidx = 4 * h + ci
                    sq7scr = hu_pool.tile([128, 512], F32, tag="sqscr", bufs=2)
                    nc.vector.scalar_tensor_tensor(
                        sq7scr[:], u7[:, usl], 1.0, u7[:, usl], ALU.mult,
                        ALU.mult, accum_out=hq7[:, cidx:cidx + 1])
            sq1 = small("sq1")
            nc.vector.tensor_reduce(sq1[:], hq7[:], axis=AX.X, op=ALU.add)
            s, b = stats_from_sums(sum7[:], sq1[:], N)
            t = hb_pool.tile([128, N], BF16, tag=f"h7b{og}")
            hs = small(f"h7s{og}")
            nc.scalar.activation(
                t[:], u7[:], AF.Prelu, bias=b[:, :], scale=s[:, :], alpha=0.2,
                accum_out=hs[:])
            h7b.append(t)
            h7sum.append(hs)
        h7sum16 = small("h7sum16", (128, 4), BF16)
        for ki in range(4):
            nc.vector.tensor_copy(h7sum16[:, ki:ki + 1], h7sum[ki][:])

        h8b = []
        for og in range(2):
            psS = pp.tile([128, HB], F32, tag="pp")
            for ki in range(4):
                nc.tensor.matmul(
                    psS[:, 0:1], w8T[ki][:, 128 * og:128 * (og + 1)],
                    h7sum16[:, ki:ki + 1], start=(ki == 0), stop=(ki == 3))
            sum8 = small("sum8")
            nc.scalar.copy(sum8[:], psS[:, 0:1])
            u8 = hu_pool.tile([128, N], F32, tag="uh", name="u8")
            hq8 = small("hq8", (128, 8))
            for ci in range(8):
                sl = slice(512 * ci, 512 * (ci + 1))
                ps8 = pp.tile([128, HB], F32, tag="pp")
                for ki in range(4):
                    nc.tensor.matmul(
                        ps8[:, 0:512],
                        w8T[ki][:, 128 * og:128 * (og + 1)],
                        h7b[ki][:, sl],
                        start=(ki == 0), stop=(ki == 3))
                nc.scalar.copy(u8[:, sl], ps8[:, 0:512])
                sq8scr = hu_pool.tile([128, 512], F32, tag="sqscr", bufs=2)
                nc.vector.scalar_tensor_tensor(
                    sq8scr[:], u8[:, sl], 1.0, u8[:, sl], ALU.mult,
                    ALU.mult, accum_out=hq8[:, ci:ci + 1])
            sq1 = small("sq1")
            nc.vector.tensor_reduce(sq1[:], hq8[:], axis=AX.X, op=ALU.add)
            s, b = stats_from_sums(sum8[:], sq1[:], N)
            t = hb_pool.tile([128, N], BF16, tag=f"h8b{og}")
            nc.scalar.activation(t[:], u8[:], AF.Prelu, bias=b[:, :], scale=s[:, :], alpha=0.2)
            h8b.append(t)

        o2 = hu_pool.tile([2, N], F32, tag="uh", name="o2")
        for ci in range(8):
            sl = slice(512 * ci, 512 * (ci + 1))
            ps9 = pp.tile([128, HB], F32, tag="pp")
            for ki in range(2):
                nc.tensor.matmul(
                    ps9[0:2, 0:512],
                    w9T[ki][:], h8b[ki][:, sl],
                    start=(ki == 0), stop=(ki == 1))
            nc.scalar.copy(o2[:, sl], ps9[0:2, 0:512])

        ost = hb_pool.tile([128, 64], F32, tag="ost")
        pst = pp.tile([128, HB], F32, tag="pp")
        for t in range(32):
            nc.tensor.transpose(
                pst[:, 2 * t:2 * (t + 1)], o2[:, 128 * t:128 * (t + 1)], id2[:])
        nc.scalar.copy(ost[:], pst[:, 0:64])
        nc.sync.dma_start(
            out_d.rearrange("(t p) c -> p t c", p=128),
            ost[:].rearrange("p (t c) -> p t c", c=2))

    nc.finalize()
    return nc


def _shared_inputs(ws):
    import ml_dtypes
    w1, w2, w3, w4, w5, w6, w7, w8, w9 = ws
    f32 = np.float32
    bf16 = ml_dtypes.bfloat16
    d = {}
    for i, w in [(1, w1), (3, w3), (5, w5)]:
        C = w.shape[1] // 2
        d[f"waT{i}"] = np.ascontiguousarray(w[:, :C].T.astype(f32))
        d[f"wbT{i}"] = np.ascontiguousarray((w[:, C:] - w[:, :C]).T.astype(f32))
    d["w2T"] = np.ascontiguousarray(w2.T.astype(f32))
    d["w4T"] = np.ascontiguousarray(w4.T.astype(f32))
    w6t = w6.T.astype(bf16); w7gt = w7[:, :1024].T.astype(bf16)
    w7xt = w7[:, 1024:].T.astype(bf16); w8t = w8.T.astype(bf16)
    w9t = w9.T.astype(bf16)
    for k in range(3):
        d[f"w6T_{k}"] = np.ascontiguousarray(w6t[64 * k:64 * (k + 1)])
        d[f"w7xT_{k}"] = np.ascontiguousarray(w7xt[64 * k:64 * (k + 1)])
    for k in range(8):
        d[f"w7gT_{k}"] = np.ascontiguousarray(w7gt[128 * k:128 * (k + 1)])
    for k in range(4):
        d[f"w8T_{k}"] = np.ascontiguousarray(w8t[128 * k:128 * (k + 1)])
    for k in range(2):
        d[f"w9T_{k}"] = np.ascontiguousarray(w9t[128 * k:128 * (k + 1)])
    d["iota256"] = np.broadcast_to(
        np.arange(CHUNK, dtype=np.uint32)[None, :], (128, CHUNK)).copy()
    d["maskc"] = np.full((128, 1), 0xFFFFFF00, dtype=np.uint32)
    d["magic"] = np.full((128, 1), 0x5F3759DF, dtype=np.uint32)
    d["id2"] = np.eye(2, dtype=f32)
    return d


def _run(inputs, want_debug=False):
    from concourse.bass_utils import run_bass_kernel_spmd

    if "nc" not in _CACHE:
        _CACHE["nc"] = _build_program()
    nc = _CACHE["nc"]

    x = np.asarray(inputs["x"], dtype=np.float32)
    ws = [np.asarray(inputs[f"w{i}"], dtype=np.float32) for i in range(1, 10)]
    shared = _shared_inputs(ws)
    in_maps = []
    for c in range(8):
        s, h = c // 2, c % 2
        xt = np.ascontiguousarray(x[s].T.astype(np.float32))
        m = dict(shared)
        m["xt"] = xt
        m["xo"] = np.ascontiguousarray(xt[:, 2048 * h:2048 * (h + 1)])
        in_maps.append(m)
    res = run_bass_kernel_spmd(nc, in_maps, list(range(8)))
    out = np.stack([res.results[2 * s]["out"] for s in range(4)])
    if want_debug:
        return out, [res.results[c] for c in range(8)]
    return out


def kernel(**inputs):
    return _run(inputs)
